# revision 26
# baseline (speedup 1.0000x reference)
"""ChebyNet (K=1) dual-branch MLP + BN kernel for 8 Trainium2 NeuronCores.

Network (per reference):
  branch b in {1,2}:  h = relu(BN(x_b @ W1_b)) ; h = relu(BN(h @ W2_b)) ; f_b = h @ Wf_b + bf_b
  out = relu(concat(f_1, f_2) @ Wh1 + bh1) @ Wh2 + bh2

ChebConv with K=1 ignores edge_index/edge_weight entirely.  Training-mode
BatchNorm over the node axis makes the linear-layer biases b1/b2 cancel
exactly, so they are never loaded.  bf_b is absorbed into bh1 on the host
(bh1' = bh1 + Wh1^T [bf_1; bf_2]), so the Lf output f is bias-free.

Sharding: nodes (axis 0) split across 8 cores, 12500 each, zero-padded to
12544 = 98*128.  Weights replicated.  BN batch stats are combined with an
AllReduce(add) of per-core (sum, sumsq) over the 8 cores; one collective
per (layer, branch), interleaved so each hides under surrounding compute.

Layouts/dtypes:
 - x arrives host-transposed as bf16 xT [feat, branch, node] (no PE
   transposes) plus a node-major fp8 copy xin8 used only for the layer-1
   Gram-matrix BN statistics (X^T X), computed with fp8 DoubleRow matmuls.
 - L1 runs in bf16; L2/Lf/Wh2 in bf16; the big head GEMM Wh1 (K=1024) runs
   in fp8e4m3 DoubleRow with 3-term error compensation:
      Wh1*64 = Whi + Wlo (host-split fp8 pair), f = fhi + flo (device split)
      t = (Whi.fhi + Whi.flo + Wlo.fhi)/64  -- residual ~1e-3 relative.
   fhi = fp8(f) on DVE (copy from PSUM), flo = fp8(psum - fhi) in a single
   fused scalar_tensor_tensor, replacing the old bias-add pass (bias is
   absorbed into bh1').  DoubleRow packs 2 k-tiles per matmul at 0.5
   cycles/row, so Wh1 costs 0.375x its bf16 cycles.
 - Layer-1 BN stats use the Gram identity: sumsq(pre1) = diag(W1^T (X^T X) W1)
   and sum(pre1) = W1^T (X^T 1), so pre1 is never materialized in pass 1.
"""

import os

os.environ.setdefault("JAX_PLATFORMS", "axon,cpu")

import numpy as np
import ml_dtypes

import concourse.bacc as bacc
import concourse.mybir as mybir
import concourse.tile as tile
from concourse import bass_utils
from concourse.bass import ts, _add_dep_helper

F32 = mybir.dt.float32
F32R = mybir.dt.float32r
BF16 = mybir.dt.bfloat16
FP8 = mybir.dt.float8e4
AF = mybir.ActivationFunctionType
ALU = mybir.AluOpType
DR = mybir.MatmulPerfMode.DoubleRow

NTOT = 100000          # true node count
NCORES = 8
NSH = NTOT // NCORES   # 12500 true nodes per core
NP = 12544             # padded per-core nodes (= 98 * 128)
NT8 = NP // 256        # 49 gram node-tile pairs
T = 512                # node-chunk size (free dim of matmuls / PSUM bank)
CHUNKS = [(i * T, T) for i in range(NP // T)] + ([(NP - NP % T, NP % T)] if NP % T else [])
C = len(CHUNKS)
XS = 4                 # xT DMA slices per branch (for xsum overlap)
PAD0 = NSH - (NP - (NP % T or T))  # first padded column inside last chunk (212)
EPS = 1e-5
PF3 = 8                # pass-3 b0 run-ahead chunks (hides AllReduce(1,1))

_CACHE = {}


def _build_program():
    nc = bacc.Bacc("TRN2", target_bir_lowering=False, debug=False,
                   num_devices=NCORES)

    # ---- kernel I/O -----------------------------------------------------
    xTd = nc.dram_tensor("XT", [128, 2, NP], BF16, kind="ExternalInput")
    x8d = [nc.dram_tensor(f"XIN8_{b+1}", [128, NT8, 256], FP8,
                          kind="ExternalInput") for b in range(2)]
    w1d = nc.dram_tensor("W1S", [128, 2, 512], F32, kind="ExternalInput")
    w2d = [nc.dram_tensor(f"W2_{b+1}", [512, 512], BF16, kind="ExternalInput")
           for b in range(2)]
    wfd = [nc.dram_tensor(f"Wf_{b+1}", [512, 512], BF16, kind="ExternalInput")
           for b in range(2)]
    gd = [[nc.dram_tensor(f"g{l+1}_{b+1}", [512], F32, kind="ExternalInput")
           for b in range(2)] for l in range(2)]
    bed = [[nc.dram_tensor(f"be{l+1}_{b+1}", [512], F32, kind="ExternalInput")
            for b in range(2)] for l in range(2)]
    whhd = nc.dram_tensor("WH1HI", [128, 8, 512], FP8, kind="ExternalInput")
    whld = nc.dram_tensor("WH1LO", [128, 8, 512], FP8, kind="ExternalInput")
    bh1d = nc.dram_tensor("BH1P", [512], F32, kind="ExternalInput")
    wh2d = nc.dram_tensor("Wh2", [512, 10], BF16, kind="ExternalInput")
    bh2d = nc.dram_tensor("bh2", [10], F32, kind="ExternalInput")
    auxd = nc.dram_tensor("AUX", [128, 4], F32R, kind="ExternalInput")
    epsd = nc.dram_tensor("EPSA", [128, 1], F32, kind="ExternalInput")
    outd = nc.dram_tensor("OUT", [10, NP], F32, kind="ExternalOutput")

    # ---- DRAM scratch ---------------------------------------------------
    spill = nc.dram_tensor("pre2_spill", [128, 2, 4, NP], BF16)
    cc_in = [[nc.dram_tensor(f"cc{l}{b}_in", [128, 4, 2], F32) for b in range(2)]
             for l in range(2)]
    cc_out = [[nc.dram_tensor(f"cc{l}{b}_out", [NCORES, 128, 4, 2], F32,
                              addr_space="Shared") for b in range(2)]
              for l in range(2)]

    def vec_ap(h, p=128):
        return h.ap().rearrange("(m p) -> p m", p=p)

    with tile.TileContext(nc) as tc:
        with (
            tc.tile_pool(name="wpool", bufs=1) as wp,
            tc.tile_pool(name="stat", bufs=1) as stat,
            tc.tile_pool(name="pf3", bufs=PF3) as pf3,
        ):
            pf3_tiles = {}
            ones_r = wp.tile([128, 4], F32R, name="ones_r")
            nc.sync.dma_start(ones_r[:], auxd[:, :])
            eps_t = stat.tile([128, 1], F32, name="eps_t")
            nc.scalar.dma_start(eps_t[:], epsd[:, :])
            # prime the ACT function tables during the input-DMA wait so the
            # first real Relu/Sqrt doesn't pay the 1.3us table load
            warm = stat.tile([128, 1], F32, name="warm")
            nc.scalar.activation(warm[:], eps_t[:], AF.Relu)
            nc.scalar.activation(warm[:], eps_t[:], AF.Sqrt, bias=eps_t[:])
            nc.scalar.activation(warm[:], eps_t[:], AF.Identity, bias=eps_t[:])

            # W1 now (pass-1 projection needs it); everything else deferred.
            w1f = wp.tile([128, 2, 512], F32, name="w1f")
            nc.scalar.dma_start(w1f[:], w1d[:, :, :])
            w1_bf, w1_r = [], []
            for b in range(2):
                w1b = wp.tile([128, 512], BF16, name=f"w1b_{b}")
                nc.vector.tensor_copy(w1b[:], w1f[:, b, :])
                w1r = wp.tile([128, 512], F32R, name=f"w1r_{b}")
                nc.vector.tensor_copy(w1r[:], w1b[:])
                w1_bf.append(w1b)
                w1_r.append(w1r)

            # tiles declared up front, DMAs emitted later via the loaders
            w2_t = [wp.tile([128, 4, 512], BF16, name=f"w2_{b}") for b in range(2)]
            wf_t = [wp.tile([128, 4, 512], BF16, name=f"wf_{b}") for b in range(2)]
            wh1h = wp.tile([128, 8, 512], FP8, name="wh1h")
            wh1l = wp.tile([128, 8, 512], FP8, name="wh1l")
            wh2_t = wp.tile([128, 4, 10], BF16, name="wh2_t")
            bh1_sb = wp.tile([128, 4], F32, name="bh1_sb")
            bh2_sb = wp.tile([10, 1], F32, name="bh2_sb")
            g_sb = [stat.tile([128, 2, 4], F32, name=f"g_sb{l}") for l in range(2)]
            be_sb = [stat.tile([128, 2, 4], F32, name=f"be_sb{l}") for l in range(2)]

            def load_pass2_weights(b):
                nc.scalar.dma_start(
                    w2_t[b][:], w2d[b].ap().rearrange("(k p) m -> p k m", p=128))
                nc.scalar.dma_start(g_sb[0][:, b, :], vec_ap(gd[0][b]))
                nc.scalar.dma_start(be_sb[0][:, b, :], vec_ap(bed[0][b]))

            def load_pass3_weights():
                for b in range(2):
                    nc.scalar.dma_start(
                        wf_t[b][:], wfd[b].ap().rearrange("(k p) m -> p k m", p=128))
                    nc.scalar.dma_start(g_sb[1][:, b, :], vec_ap(gd[1][b]))
                    nc.scalar.dma_start(be_sb[1][:, b, :], vec_ap(bed[1][b]))
                nc.scalar.dma_start(wh1h[:], whhd[:, :, :])
                nc.scalar.dma_start(wh1l[:], whld[:, :, :])
                nc.scalar.dma_start(
                    wh2_t[:], wh2d.ap().rearrange("(k p) m -> p k m", p=128))
                nc.scalar.dma_start(bh1_sb[:], vec_ap(bh1d))
                nc.scalar.dma_start(bh2_sb[:],
                                    bh2d.ap().rearrange("(m o) -> m o", o=1))

            st2 = stat.tile([128, 2, 4, C, 6], F32, name="st2")
            pay = [[stat.tile([128, 4, 2], F32, name=f"pay{l}{b}")
                    for b in range(2)] for l in range(2)]
            scale_t = [stat.tile([128, 2, 4], F32, name=f"scale{l}") for l in range(2)]
            shift_t = [stat.tile([128, 2, 4], F32, name=f"shift{l}") for l in range(2)]

            pay_dma = {}

            def issue_allreduce(l, b):
                # payload on the SWDGE queue: never queues behind bulk input
                # or spill DMAs, so the collective launches immediately.
                # AllGather + local sum is ~2x faster than AllReduce.
                pay_dma[(l, b)] = nc.gpsimd.dma_start(
                    cc_in[l][b][:, :, :], pay[l][b][:])
                nc.gpsimd.collective_compute(
                    "AllGather", mybir.AluOpType.bypass,
                    replica_groups=[list(range(NCORES))],
                    ins=[cc_in[l][b].ap().opt()], outs=[cc_out[l][b].ap().opt()],
                )

            gl_tiles = {}

            def load_stats(l, b, dma_engine=None):
                gl = stat.tile([128, NCORES, 4, 2], F32, tag=f"gl{l}{b}",
                               name=f"gl{l}{b}")
                (dma_engine or nc.sync).dma_start(
                    gl[:], cc_out[l][b].ap().rearrange("c p m s -> p c m s"))
                gl_tiles[(l, b)] = gl

            def finish_stats(l, b):
                """cc_out[l][b] -> scale_t[l][:, b, :], shift_t[l][:, b, :]."""
                if (l, b) not in gl_tiles:
                    load_stats(l, b)
                glg = gl_tiles.pop((l, b))
                # sum the 8 gathered per-core payloads (3-level tree)
                s4 = stat.tile([128, 4, 4, 2], F32, tag="s4", name=f"s4_{l}{b}")
                nc.vector.tensor_add(s4[:], glg[:, 0:4, :, :], glg[:, 4:8, :, :])
                s2 = stat.tile([128, 2, 4, 2], F32, tag="s2", name=f"s2_{l}{b}")
                nc.vector.tensor_add(s2[:], s4[:, 0:2, :, :], s4[:, 2:4, :, :])
                gl = stat.tile([128, 4, 2], F32, tag=f"gls{l}{b}",
                               name=f"gls{l}{b}")
                nc.vector.tensor_add(gl[:], s2[:, 0, :, :], s2[:, 1, :, :])
                mu = stat.tile([128, 4], F32, tag="mu", name=f"mu{l}{b}")
                var = stat.tile([128, 4], F32, tag="var", name=f"var{l}{b}")
                tmp = stat.tile([128, 4], F32, tag="tmpf", name=f"tmp{l}{b}")
                nc.vector.tensor_scalar_mul(mu[:], gl[:, :, 0], 1.0 / NTOT)
                nc.vector.tensor_scalar_mul(var[:], gl[:, :, 1], 1.0 / NTOT)
                nc.vector.tensor_mul(tmp[:], mu[:], mu[:])
                nc.vector.tensor_sub(var[:], var[:], tmp[:])
                nc.scalar.activation(var[:], var[:], AF.Sqrt, bias=eps_t[:])
                nc.vector.reciprocal(var[:], var[:])
                nc.vector.tensor_mul(scale_t[l][:, b, :], g_sb[l][:, b, :], var[:])
                nc.vector.tensor_mul(tmp[:], mu[:], scale_t[l][:, b, :])
                nc.vector.tensor_sub(shift_t[l][:, b, :], be_sb[l][:, b, :], tmp[:])

            # ================= passes 1+2 (share the resident xT) ========
            with tc.tile_pool(name="xtp", bufs=1) as xtp:
              # resident transposed input, bf16: [feat, branch, node]
              xT = xtp.tile([128, 2, NP], BF16, name="xT")

              # ---- pass 1: DMA + fp8 Gram stats ----
              with (
                tc.tile_pool(name="w1p", bufs=2) as w1p,
                tc.tile_pool(name="ps_g", bufs=1, space="PSUM") as ps_g,
                tc.tile_pool(name="ps_pj", bufs=1, space="PSUM") as ps_pj,
              ):
                  XSL = NP // XS
                  ones8 = wp.tile([128, 2, 1], FP8, name="ones8")
                  nc.vector.tensor_copy(ones8[:], ones_r[:, 0:2].rearrange(
                      "p (s o) -> p s o", o=1))
                  NH = NT8 // 2

                  def x8_dma(b, lo, hi):
                      return nc.sync.dma_start(
                          x8t[b][:, lo:hi, :, :],
                          x8d[b].ap()[:, lo:hi, :].rearrange(
                              "p t (s f) -> p t s f", s=2))

                  x8t = [w1p.tile([128, NT8, 2, 128], FP8, name=f"x8_{b}")
                         for b in range(2)]
                  # only x8_0 is on the AR(0,0) critical path: everything else
                  # is deferred until the AR payload has won the DMA queue
                  x8_dma(0, 0, NH)
                  x8_dma(0, NH, NT8)
                  for b in range(2):
                      # Gram X^T X and colsum X^T 1, both via fp8 DoubleRow
                      g_ps = ps_g.tile([128, 128], F32, tag="G", name=f"G_{b}")
                      cs_ps = ps_g.tile([128, 1], F32, tag="CS", name=f"CS_{b}")
                      for t in range(NT8):
                          nc.tensor.matmul(g_ps[:], x8t[b][:, t, :, :],
                                           x8t[b][:, t, :, :],
                                           start=(t == 0), stop=(t == NT8 - 1),
                                           perf_mode=DR)
                          nc.tensor.matmul(cs_ps[:], x8t[b][:, t, :, :],
                                           ones8[:],
                                           start=(t == 0), stop=(t == NT8 - 1),
                                           perf_mode=DR)
                      # ---- project Gram -> (sum, sumsq) of pre1 ----
                      g_sbuf = w1p.tile([128, 128], F32R, tag="gsb", name=f"gsb_{b}")
                      nc.vector.tensor_copy(g_sbuf[:], g_ps[:])
                      mm1 = ps_pj.tile([128, 512], F32, tag="pj", name=f"mm1_{b}")
                      nc.tensor.matmul(mm1[:], g_sbuf[:], w1_r[b][:], start=True,
                                       stop=True)
                      mm1_sb = w1p.tile([128, 512], F32R, tag="mm1sb",
                                        name=f"mm1sb_{b}")
                      nc.vector.tensor_copy(mm1_sb[:], mm1[:])
                      prod = w1p.tile([128, 512], F32R, tag="prod", name=f"prod_{b}")
                      nc.vector.tensor_mul(prod[:], w1_r[b][:], mm1_sb[:])
                      xsum_r = w1p.tile([128, 4], F32R, tag="xsumr",
                                        name=f"xsumr_{b}")
                      for q in range(4):
                          nc.vector.tensor_copy(xsum_r[:, q:q + 1], cs_ps[:])
                      for m in range(4):
                          sq = ps_pj.tile([128, 4], F32, tag="pj2", name=f"sq_{b}_{m}")
                          nc.tensor.matmul(sq[:], prod[:, ts(m, 128)], ones_r[:],
                                           start=True, stop=True)
                          nc.vector.tensor_copy(pay[0][b][:, m, 1:2], sq[:, 0:1])
                          sm = ps_pj.tile([128, 4], F32, tag="pj2", name=f"sm_{b}_{m}")
                          nc.tensor.matmul(sm[:], w1_r[b][:, ts(m, 128)], xsum_r[:],
                                           start=True, stop=True)
                          nc.vector.tensor_copy(pay[0][b][:, m, 0:1], sm[:, 0:1])
                      issue_allreduce(0, b)
                      load_pass2_weights(b)
                      if b == 0:
                          # bulk inputs, gated behind the AR(0,0) payload DMA
                          gate = pay_dma[(0, 0)].ins
                          bulk = [x8_dma(1, 0, NH), x8_dma(1, NH, NT8)]
                          for b2 in range(2):
                              for s in range(XS):
                                  bulk.append(nc.sync.dma_start(
                                      xT[:, b2, s * XSL:(s + 1) * XSL],
                                      xTd[:, b2, s * XSL:(s + 1) * XSL]))
                          for d in bulk:
                              _add_dep_helper(d.ins, gate, sync=True,
                                              reason="bulk after AR payload")

              # ================= pass 2: L1 -> BN1 -> L2 -> stats/spill ====
              with (
                  tc.tile_pool(name="w2p", bufs=4) as w2p,
                  tc.tile_pool(name="ps_p1", bufs=4, space="PSUM") as ps_p1,
                  tc.tile_pool(name="ps_p2", bufs=3, space="PSUM") as ps_p2,
              ):
                  for b in range(2):
                      finish_stats(0, b)
                      for c, (c0, tc_sz) in enumerate(CHUNKS):
                          if b == 1 and c == 6:
                              # AR(1,0) is long done by now; computing its
                              # scale/shift here keeps pass-3 startup off the
                              # critical path
                              finish_stats(1, 0)
                          h1 = w2p.tile([128, 4, tc_sz], BF16, tag="h1",
                                        name=f"h1_{c}_{b}")
                          for m in range(4):
                              pp = ps_p1.tile([128, tc_sz], F32, tag="p1",
                                              name=f"p1_{c}_{b}_{m}")
                              nc.tensor.matmul(pp[:], w1_bf[b][:, ts(m, 128)],
                                               xT[:, b, c0:c0 + tc_sz],
                                               start=True, stop=True)
                              nc.scalar.activation(
                                  h1[:, m, :], pp[:], AF.Relu,
                                  bias=shift_t[0][:, b, m:m + 1],
                                  scale=scale_t[0][:, b, m:m + 1])
                          if c == C - 1:
                              # padded nodes: relu(shift) != 0 would pollute BN2 stats
                              nc.scalar.mul(h1[:, :, PAD0:], h1[:, :, PAD0:], 0.0)
                          spl = w2p.tile([128, 4, tc_sz], BF16, tag="spl",
                                         name=f"spl_{c}_{b}")
                          for m in range(4):
                              pq = ps_p2.tile([128, tc_sz], F32, tag="p2",
                                              name=f"p2_{c}_{b}_{m}")
                              for k in range(4):
                                  nc.tensor.matmul(pq[:], w2_t[b][:, k, ts(m, 128)],
                                                   h1[:, k, :],
                                                   start=(k == 0), stop=(k == 3))
                              # copy frees the PSUM bank; stats read the SBUF
                              # copy and can lag without stalling the PE.
                              # Last chunks go all-DVE so ACT is free to start
                              # pass-3's h2 immediately.
                              if m % 2 == 0 and not (b == 1 and c >= C - 2):
                                  nc.scalar.copy(spl[:, m, :], pq[:])
                              else:
                                  nc.vector.tensor_copy(spl[:, m, :], pq[:])
                          for m in range(4):
                              nc.vector.bn_stats(st2[:, b, m, c, :], spl[:, m, :])
                          nc.sync.dma_start(spill[:, b, :, c0:c0 + tc_sz], spl[:])
                      # ---- aggregate local BN2 stats, launch AllReduce ----
                      agg = stat.tile([128, 4, 2], F32, tag="agg", name=f"agg_{b}")
                      for m in range(4):
                          nc.vector.bn_aggr(agg[:, m, :], st2[:, b, m, :, :])
                      tmp2 = stat.tile([128, 4], F32, tag="tmp2", name=f"tmp2_{b}")
                      nc.vector.tensor_scalar_mul(pay[1][b][:, :, 0], agg[:, :, 0],
                                                  float(NP))
                      nc.vector.tensor_mul(tmp2[:], agg[:, :, 0], agg[:, :, 0])
                      nc.vector.tensor_add(tmp2[:], tmp2[:], agg[:, :, 1])
                      nc.vector.tensor_scalar_mul(pay[1][b][:, :, 1], tmp2[:],
                                                  float(NP))
                      issue_allreduce(1, b)
                      if b == 0:
                          load_pass3_weights()
                          # prefetch first b0 spill chunks on the gpsimd
                          # SWDGE queue (drains right after AR(1,0)), so
                          # pass-3 b0 sections can run during AR(1,1)
                          for cq in range(PF3):
                              cq0, cqs = CHUNKS[cq]
                              pftile = pf3.tile([128, 4, cqs], BF16, tag="pf",
                                                name=f"pf3_{cq}")
                              nc.gpsimd.dma_start(
                                  pftile[:], spill[:, 0, :, cq0:cq0 + cqs])
                              pf3_tiles[cq] = pftile
                          load_stats(1, 0, dma_engine=nc.gpsimd)

            # ================= pass 3: BN2 -> Lf -> fp8 head =============
            with (
                tc.tile_pool(name="w3p", bufs=2) as w3p,
                tc.tile_pool(name="ps_f", bufs=4, space="PSUM") as ps_f,
                tc.tile_pool(name="ps_t", bufs=4, space="PSUM") as ps_t,
            ):
                ps_o = ps_t  # share the t/o banks (tag-separated slots share pool)

                f_tiles = {}

                def b_section(c, b):
                    """BN2+relu -> Lf -> fp8 hi/lo split of f for branch b."""
                    c0, tc_sz = CHUNKS[c]
                    fhi = w3p.tile([128, 4, tc_sz], FP8, tag=f"fhi{b}",
                                   bufs=(PF3 + 1 if b == 0 else 2),
                                   name=f"fhi{b}_{c}")
                    flo = w3p.tile([128, 4, tc_sz], FP8, tag=f"flo{b}",
                                   bufs=(PF3 + 1 if b == 0 else 2),
                                   name=f"flo{b}_{c}")
                    if b == 0:
                        f_tiles[c] = (fhi, flo)
                    if b == 0 and c in pf3_tiles:
                        pre2 = pf3_tiles.pop(c)
                    else:
                        pre2 = w3p.tile([128, 4, tc_sz], BF16, tag="pre2ld",
                                        bufs=4, name=f"pre2_{c}_{b}")
                        nc.sync.dma_start(pre2[:], spill[:, b, :, c0:c0 + tc_sz])
                    h2 = w3p.tile([128, 4, tc_sz], BF16, tag="h2", bufs=3,
                                  name=f"h2_{c}_{b}")
                    for k in range(4):
                        nc.scalar.activation(
                            h2[:, k, :], pre2[:, k, :], AF.Relu,
                            bias=shift_t[1][:, b, k:k + 1],
                            scale=scale_t[1][:, b, k:k + 1])
                    for m in range(4):
                        pf = ps_f.tile([128, tc_sz], F32, tag="f",
                                       name=f"pf_{c}_{b}_{m}")
                        for k in range(4):
                            nc.tensor.matmul(pf[:], wf_t[b][:, k, ts(m, 128)],
                                             h2[:, k, :],
                                             start=(k == 0), stop=(k == 3))
                        # split hi-copies between ACT and DVE so neither
                        # engine paces the (DVE-heavy) f-split
                        if b == 0 and m % 2 == 0:
                            nc.scalar.copy(fhi[:, m, :], pf[:])
                        else:
                            nc.vector.tensor_copy(fhi[:, m, :], pf[:])
                        nc.vector.scalar_tensor_tensor(
                            flo[:, m, :], pf[:], 1.0, fhi[:, m, :],
                            op0=ALU.mult, op1=ALU.subtract)
                    return fhi, flo

                # run-ahead: b0 sections of the prefetched chunks execute
                # while AllReduce (1,1) is still in flight
                for c in range(PF3):
                    b_section(c, 0)
                    if c == 5:
                        finish_stats(1, 1)

                for c, (c0, tc_sz) in enumerate(CHUNKS):
                    if c >= PF3:
                        b_section(c, 0)
                    f0hi, f0lo = f_tiles.pop(c)
                    f1hi, f1lo = b_section(c, 1)
                    # ---- head: Wh1 fp8 DoubleRow 3-term ----
                    t_sb = w3p.tile([128, 4, tc_sz], BF16, tag="t_sb", bufs=2,
                                    name=f"t_sb_{c}")
                    for m in range(4):
                        ptl = ps_t.tile([128, tc_sz], F32, tag="t",
                                        name=f"ptl_{c}_{m}")
                        n_mm = 0
                        for fh, fl, kb in ((f0hi, f0lo, 0), (f1hi, f1lo, 4)):
                            for j in range(2):
                                wsl_h = wh1h[:, kb + 2 * j:kb + 2 * j + 2, ts(m, 128)]
                                wsl_l = wh1l[:, kb + 2 * j:kb + 2 * j + 2, ts(m, 128)]
                                fsl_h = fh[:, 2 * j:2 * j + 2, :]
                                fsl_l = fl[:, 2 * j:2 * j + 2, :]
                                for wsl, fsl in ((wsl_h, fsl_h), (wsl_h, fsl_l),
                                                 (wsl_l, fsl_h)):
                                    nc.tensor.matmul(ptl[:], wsl, fsl,
                                                     start=(n_mm == 0),
                                                     stop=(n_mm == 11),
                                                     perf_mode=DR)
                                    n_mm += 1
                        nc.scalar.activation(t_sb[:, m, :], ptl[:], AF.Relu,
                                             bias=bh1_sb[:, m:m + 1],
                                             scale=1.0 / 64.0)
                    po = ps_o.tile([10, tc_sz], F32, tag="t", name=f"po_{c}")
                    for k in range(4):
                        nc.tensor.matmul(po[:], wh2_t[:, k, :], t_sb[:, k, :],
                                         start=(k == 0), stop=(k == 3))
                    o_sb = w3p.tile([10, tc_sz], F32, tag="o_sb", name=f"o_sb_{c}")
                    nc.scalar.activation(o_sb[:], po[:], AF.Identity,
                                         bias=bh2_sb[:, 0:1])
                    nc.sync.dma_start(outd[:, c0:c0 + tc_sz], o_sb[:])

    nc.compile()
    return nc


def _get_program():
    if "nc" not in _CACHE:
        _CACHE["nc"] = _build_program()
    return _CACHE["nc"]


def kernel(**inputs):
    nc = _get_program()
    F8 = ml_dtypes.float8_e4m3
    B16 = ml_dtypes.bfloat16

    def shard_pad(x):
        x = np.ascontiguousarray(x, dtype=np.float32).reshape(NCORES, NSH, 128)
        pad = np.zeros((NCORES, NP - NSH, 128), dtype=np.float32)
        return np.concatenate([x, pad], axis=1)  # [NCORES, NP, 128]

    xp = [shard_pad(inputs["x_1"]), shard_pad(inputs["x_2"])]
    # xT: [NCORES, 128, 2, NP] bf16
    xT = np.stack([np.swapaxes(xp[0], 1, 2), np.swapaxes(xp[1], 1, 2)],
                  axis=1).astype(B16)  # [NCORES, 2, 128, NP]
    xT = np.ascontiguousarray(np.swapaxes(xT, 1, 2))  # [NCORES, 128, 2, NP]
    # xin8: [NCORES, 128, NT8, 256] fp8 per branch (node-within-tile on the
    # partition axis, contiguous per partition row for fast DMA)
    x8 = [np.ascontiguousarray(
              xp[b].reshape(NCORES, NT8, 2, 128, 128).transpose(0, 3, 1, 2, 4)
              .reshape(NCORES, 128, NT8, 256)).astype(F8)
          for b in range(2)]

    rep = {}
    # W1 stacked [128, 2, 512]
    rep["W1S"] = np.ascontiguousarray(
        np.stack([inputs["W1_1"], inputs["W1_2"]], axis=1), dtype=np.float32)
    for nm in ("W2_1", "W2_2", "Wf_1", "Wf_2"):
        rep[nm] = np.ascontiguousarray(inputs[nm]).astype(B16)
    rep["Wh2"] = np.ascontiguousarray(inputs["Wh2"]).astype(B16)
    for nm in ("g1_1", "be1_1", "g2_1", "be2_1",
               "g1_2", "be1_2", "g2_2", "be2_2", "bh2"):
        rep[nm] = np.ascontiguousarray(inputs[nm], dtype=np.float32)

    # Wh1 -> x64 fp8 hi/lo pair in [128, 8, 512] (p k m) layout
    wh1 = np.ascontiguousarray(inputs["Wh1"], dtype=np.float32) * 64.0
    wh1_pkm = wh1.reshape(8, 128, 512).swapaxes(0, 1)  # [128, 8, 512]
    whi = wh1_pkm.astype(F8)
    wlo = (wh1_pkm - whi.astype(np.float32)).astype(F8)
    rep["WH1HI"] = np.ascontiguousarray(whi)
    rep["WH1LO"] = np.ascontiguousarray(wlo)
    # bh1' = bh1 + Wh1^T [bf_1; bf_2]  (absorbs the Lf biases)
    bfcat = np.concatenate([np.asarray(inputs["bf_1"], np.float64),
                            np.asarray(inputs["bf_2"], np.float64)])
    rep["BH1P"] = (np.asarray(inputs["bh1"], np.float64)
                   + bfcat @ np.asarray(inputs["Wh1"], np.float64)).astype(np.float32)

    rep["AUX"] = np.ones((128, 4), dtype=np.float32)
    rep["EPSA"] = np.full((128, 1), EPS, dtype=np.float32)

    in_maps = []
    for c in range(NCORES):
        m = {"XT": xT[c], "XIN8_1": x8[0][c], "XIN8_2": x8[1][c]}
        m.update(rep)
        in_maps.append(m)

    res = bass_utils.run_bass_kernel_spmd(nc, in_maps, core_ids=list(range(NCORES)))
    parts = [res.results[c]["OUT"][:, :NSH] for c in range(NCORES)]
    out = np.concatenate(parts, axis=1).T
    return np.ascontiguousarray(out, dtype=np.float32)


# revision 30
# speedup vs baseline: 1.0023x; 1.0023x over previous
"""ChebyNet (K=1) dual-branch MLP + BN kernel for 8 Trainium2 NeuronCores.

Network (per reference):
  branch b in {1,2}:  h = relu(BN(x_b @ W1_b)) ; h = relu(BN(h @ W2_b)) ; f_b = h @ Wf_b + bf_b
  out = relu(concat(f_1, f_2) @ Wh1 + bh1) @ Wh2 + bh2

ChebConv with K=1 ignores edge_index/edge_weight entirely.  Training-mode
BatchNorm over the node axis makes the linear-layer biases b1/b2 cancel
exactly, so they are never loaded.  bf_b is absorbed into bh1 on the host
(bh1' = bh1 + Wh1^T [bf_1; bf_2]), so the Lf output f is bias-free.

Sharding: nodes (axis 0) split across 8 cores, 12500 each, zero-padded to
12544 = 98*128.  Weights replicated.  BN batch stats are combined with an
AllReduce(add) of per-core (sum, sumsq) over the 8 cores; one collective
per (layer, branch), interleaved so each hides under surrounding compute.

Layouts/dtypes:
 - x arrives host-transposed as bf16 xT [feat, branch, node] (no PE
   transposes) plus a node-major fp8 copy xin8 used only for the layer-1
   Gram-matrix BN statistics (X^T X), computed with fp8 DoubleRow matmuls.
 - L1 runs in bf16; L2/Lf/Wh2 in bf16; the big head GEMM Wh1 (K=1024) runs
   in fp8e4m3 DoubleRow with 3-term error compensation:
      Wh1*64 = Whi + Wlo (host-split fp8 pair), f = fhi + flo (device split)
      t = (Whi.fhi + Whi.flo + Wlo.fhi)/64  -- residual ~1e-3 relative.
   fhi = fp8(f) on DVE (copy from PSUM), flo = fp8(psum - fhi) in a single
   fused scalar_tensor_tensor, replacing the old bias-add pass (bias is
   absorbed into bh1').  DoubleRow packs 2 k-tiles per matmul at 0.5
   cycles/row, so Wh1 costs 0.375x its bf16 cycles.
 - Layer-1 BN stats use the Gram identity: sumsq(pre1) = diag(W1^T (X^T X) W1)
   and sum(pre1) = W1^T (X^T 1), so pre1 is never materialized in pass 1.
"""

import os

os.environ.setdefault("JAX_PLATFORMS", "axon,cpu")

import numpy as np
import ml_dtypes

import concourse.bacc as bacc
import concourse.mybir as mybir
import concourse.tile as tile
from concourse import bass_utils
from concourse.bass import ts, _add_dep_helper

F32 = mybir.dt.float32
F32R = mybir.dt.float32r
BF16 = mybir.dt.bfloat16
FP8 = mybir.dt.float8e4
AF = mybir.ActivationFunctionType
ALU = mybir.AluOpType
DR = mybir.MatmulPerfMode.DoubleRow

NTOT = 100000          # true node count
NCORES = 8
NSH = NTOT // NCORES   # 12500 true nodes per core
NP = 12544             # padded per-core nodes (= 98 * 128)
NT8 = NP // 256        # 49 gram node-tile pairs
T = 512                # node-chunk size (free dim of matmuls / PSUM bank)
CHUNKS = [(i * T, T) for i in range(NP // T)] + ([(NP - NP % T, NP % T)] if NP % T else [])
C = len(CHUNKS)
XS = 4                 # xT DMA slices per branch (for xsum overlap)
PAD0 = NSH - (NP - (NP % T or T))  # first padded column inside last chunk (212)
EPS = 1e-5
PF3 = 8                # pass-3 b0 run-ahead chunks (hides AllReduce(1,1))

_CACHE = {}


def _build_program():
    nc = bacc.Bacc("TRN2", target_bir_lowering=False, debug=False,
                   num_devices=NCORES)

    # ---- kernel I/O -----------------------------------------------------
    xTd = nc.dram_tensor("XT", [128, 2, NP], BF16, kind="ExternalInput")
    x8d = [nc.dram_tensor(f"XIN8_{b+1}", [128, NT8, 256], FP8,
                          kind="ExternalInput") for b in range(2)]
    w1d = nc.dram_tensor("W1S", [128, 2, 512], F32, kind="ExternalInput")
    w2d = [nc.dram_tensor(f"W2_{b+1}", [512, 512], BF16, kind="ExternalInput")
           for b in range(2)]
    wfd = [nc.dram_tensor(f"Wf_{b+1}", [512, 512], BF16, kind="ExternalInput")
           for b in range(2)]
    gd = [[nc.dram_tensor(f"g{l+1}_{b+1}", [512], F32, kind="ExternalInput")
           for b in range(2)] for l in range(2)]
    bed = [[nc.dram_tensor(f"be{l+1}_{b+1}", [512], F32, kind="ExternalInput")
            for b in range(2)] for l in range(2)]
    whhd = nc.dram_tensor("WH1HI", [128, 8, 512], FP8, kind="ExternalInput")
    whld = nc.dram_tensor("WH1LO", [128, 8, 512], FP8, kind="ExternalInput")
    bh1d = nc.dram_tensor("BH1P", [512], F32, kind="ExternalInput")
    wh2d = nc.dram_tensor("Wh2", [512, 10], BF16, kind="ExternalInput")
    bh2d = nc.dram_tensor("bh2", [10], F32, kind="ExternalInput")
    auxd = nc.dram_tensor("AUX", [128, 4], F32R, kind="ExternalInput")
    epsd = nc.dram_tensor("EPSA", [128, 1], F32, kind="ExternalInput")
    outd = nc.dram_tensor("OUT", [10, NP], F32, kind="ExternalOutput")

    # ---- DRAM scratch ---------------------------------------------------
    spill = nc.dram_tensor("pre2_spill", [128, 2, 4, NP], BF16)
    cc_in = [[nc.dram_tensor(f"cc{l}{b}_in", [128, 4, 2], F32) for b in range(2)]
             for l in range(2)]
    cc_out = [[nc.dram_tensor(f"cc{l}{b}_out", [NCORES, 128, 4, 2], F32,
                              addr_space="Shared") for b in range(2)]
              for l in range(2)]

    def vec_ap(h, p=128):
        return h.ap().rearrange("(m p) -> p m", p=p)

    with tile.TileContext(nc) as tc:
        with (
            tc.tile_pool(name="wpool", bufs=1) as wp,
            tc.tile_pool(name="stat", bufs=1) as stat,
            tc.tile_pool(name="pf3", bufs=PF3) as pf3,
        ):
            pf3_tiles = {}
            ones_r = wp.tile([128, 4], F32R, name="ones_r")
            nc.sync.dma_start(ones_r[:], auxd[:, :])
            eps_t = stat.tile([128, 1], F32, name="eps_t")
            nc.scalar.dma_start(eps_t[:], epsd[:, :])
            # prime the ACT function tables during the input-DMA wait so the
            # first real Relu/Sqrt doesn't pay the 1.3us table load
            warm = stat.tile([128, 1], F32, name="warm")
            nc.scalar.activation(warm[:], eps_t[:], AF.Relu)
            nc.scalar.activation(warm[:], eps_t[:], AF.Sqrt, bias=eps_t[:])
            nc.scalar.activation(warm[:], eps_t[:], AF.Identity, bias=eps_t[:])

            # W1 now (pass-1 projection needs it); everything else deferred.
            w1f = wp.tile([128, 2, 512], F32, name="w1f")
            nc.scalar.dma_start(w1f[:], w1d[:, :, :])
            w1_bf, w1_r = [], []
            for b in range(2):
                w1b = wp.tile([128, 512], BF16, name=f"w1b_{b}")
                nc.vector.tensor_copy(w1b[:], w1f[:, b, :])
                w1r = wp.tile([128, 512], F32R, name=f"w1r_{b}")
                nc.vector.tensor_copy(w1r[:], w1b[:])
                w1_bf.append(w1b)
                w1_r.append(w1r)

            # tiles declared up front, DMAs emitted later via the loaders
            w2_t = [wp.tile([128, 4, 512], BF16, name=f"w2_{b}") for b in range(2)]
            wf_t = [wp.tile([128, 4, 512], BF16, name=f"wf_{b}") for b in range(2)]
            wh1h = wp.tile([128, 8, 512], FP8, name="wh1h")
            wh1l = wp.tile([128, 8, 512], FP8, name="wh1l")
            wh2_t = wp.tile([128, 4, 10], BF16, name="wh2_t")
            bh1_sb = wp.tile([128, 4], F32, name="bh1_sb")
            bh2_sb = wp.tile([10, 1], F32, name="bh2_sb")
            g_sb = [stat.tile([128, 2, 4], F32, name=f"g_sb{l}") for l in range(2)]
            be_sb = [stat.tile([128, 2, 4], F32, name=f"be_sb{l}") for l in range(2)]

            def load_pass2_weights(b, gate):
                ds = [nc.scalar.dma_start(
                    w2_t[b][:], w2d[b].ap().rearrange("(k p) m -> p k m", p=128)),
                    nc.scalar.dma_start(g_sb[0][:, b, :], vec_ap(gd[0][b])),
                    nc.scalar.dma_start(be_sb[0][:, b, :], vec_ap(bed[0][b]))]
                for d in ds:
                    _add_dep_helper(d.ins, gate, sync=True,
                                    reason="weights after AR payload")

            def load_pass3_weights(gate):
                ds = []
                for b in range(2):
                    ds.append(nc.scalar.dma_start(
                        wf_t[b][:], wfd[b].ap().rearrange("(k p) m -> p k m", p=128)))
                    ds.append(nc.scalar.dma_start(g_sb[1][:, b, :], vec_ap(gd[1][b])))
                    ds.append(nc.scalar.dma_start(be_sb[1][:, b, :], vec_ap(bed[1][b])))
                ds.append(nc.scalar.dma_start(wh1h[:], whhd[:, :, :]))
                ds.append(nc.scalar.dma_start(wh1l[:], whld[:, :, :]))
                ds.append(nc.scalar.dma_start(
                    wh2_t[:], wh2d.ap().rearrange("(k p) m -> p k m", p=128)))
                ds.append(nc.scalar.dma_start(bh1_sb[:], vec_ap(bh1d)))
                ds.append(nc.scalar.dma_start(
                    bh2_sb[:], bh2d.ap().rearrange("(m o) -> m o", o=1)))
                for d in ds:
                    _add_dep_helper(d.ins, gate, sync=True,
                                    reason="pass3 weights after bulk input")

            st2 = stat.tile([128, 2, 4, C, 6], F32, name="st2")
            pay = [[stat.tile([128, 4, 2], F32, name=f"pay{l}{b}")
                    for b in range(2)] for l in range(2)]
            scale_t = [stat.tile([128, 2, 4], F32, name=f"scale{l}") for l in range(2)]
            shift_t = [stat.tile([128, 2, 4], F32, name=f"shift{l}") for l in range(2)]

            pay_dma = {}

            def issue_allreduce(l, b):
                # payload on the SWDGE queue: never queues behind bulk input
                # or spill DMAs, so the collective launches immediately.
                # AllGather + local sum is ~2x faster than AllReduce.
                pay_dma[(l, b)] = nc.gpsimd.dma_start(
                    cc_in[l][b][:, :, :], pay[l][b][:])
                nc.gpsimd.collective_compute(
                    "AllGather", mybir.AluOpType.bypass,
                    replica_groups=[list(range(NCORES))],
                    ins=[cc_in[l][b].ap().opt()], outs=[cc_out[l][b].ap().opt()],
                )

            gl_tiles = {}

            def load_stats(l, b, dma_engine=None):
                gl = stat.tile([128, NCORES, 4, 2], F32, tag=f"gl{l}{b}",
                               name=f"gl{l}{b}")
                (dma_engine or nc.sync).dma_start(
                    gl[:], cc_out[l][b].ap().rearrange("c p m s -> p c m s"))
                gl_tiles[(l, b)] = gl

            def finish_stats(l, b):
                """cc_out[l][b] -> scale_t[l][:, b, :], shift_t[l][:, b, :]."""
                if (l, b) not in gl_tiles:
                    load_stats(l, b)
                glg = gl_tiles.pop((l, b))
                # sum the 8 gathered per-core payloads (3-level tree)
                s4 = stat.tile([128, 4, 4, 2], F32, tag="s4", name=f"s4_{l}{b}")
                nc.vector.tensor_add(s4[:], glg[:, 0:4, :, :], glg[:, 4:8, :, :])
                s2 = stat.tile([128, 2, 4, 2], F32, tag="s2", name=f"s2_{l}{b}")
                nc.vector.tensor_add(s2[:], s4[:, 0:2, :, :], s4[:, 2:4, :, :])
                gl = stat.tile([128, 4, 2], F32, tag=f"gls{l}{b}",
                               name=f"gls{l}{b}")
                nc.vector.tensor_add(gl[:], s2[:, 0, :, :], s2[:, 1, :, :])
                mu = stat.tile([128, 4], F32, tag="mu", name=f"mu{l}{b}")
                var = stat.tile([128, 4], F32, tag="var", name=f"var{l}{b}")
                tmp = stat.tile([128, 4], F32, tag="tmpf", name=f"tmp{l}{b}")
                nc.vector.tensor_scalar_mul(mu[:], gl[:, :, 0], 1.0 / NTOT)
                nc.vector.tensor_scalar_mul(var[:], gl[:, :, 1], 1.0 / NTOT)
                nc.vector.tensor_mul(tmp[:], mu[:], mu[:])
                nc.vector.tensor_sub(var[:], var[:], tmp[:])
                nc.scalar.activation(var[:], var[:], AF.Sqrt, bias=eps_t[:])
                nc.vector.reciprocal(var[:], var[:])
                nc.vector.tensor_mul(scale_t[l][:, b, :], g_sb[l][:, b, :], var[:])
                nc.vector.tensor_mul(tmp[:], mu[:], scale_t[l][:, b, :])
                nc.vector.tensor_sub(shift_t[l][:, b, :], be_sb[l][:, b, :], tmp[:])

            # ================= passes 1+2 (share the resident xT) ========
            with tc.tile_pool(name="xtp", bufs=1) as xtp:
              # resident transposed input, bf16: [feat, branch, node]
              xT = xtp.tile([128, 2, NP], BF16, name="xT")

              # ---- pass 1: DMA + fp8 Gram stats ----
              with (
                tc.tile_pool(name="w1p", bufs=2) as w1p,
                tc.tile_pool(name="ps_g", bufs=1, space="PSUM") as ps_g,
                tc.tile_pool(name="ps_pj", bufs=1, space="PSUM") as ps_pj,
              ):
                  XSL = NP // XS
                  ones8 = wp.tile([128, 2, 1], FP8, name="ones8")
                  nc.vector.tensor_copy(ones8[:], ones_r[:, 0:2].rearrange(
                      "p (s o) -> p s o", o=1))
                  NH = NT8 // 2

                  def x8_dma(b, lo, hi):
                      return nc.sync.dma_start(
                          x8t[b][:, lo:hi, :, :],
                          x8d[b].ap()[:, lo:hi, :].rearrange(
                              "p t (s f) -> p t s f", s=2))

                  x8t = [w1p.tile([128, NT8, 2, 128], FP8, name=f"x8_{b}")
                         for b in range(2)]
                  # only x8_0 is on the AR(0,0) critical path: everything else
                  # is deferred until the AR payload has won the DMA queue
                  x8_dma(0, 0, NH)
                  x8_dma(0, NH, NT8)
                  for b in range(2):
                      # Gram X^T X and colsum X^T 1, both via fp8 DoubleRow
                      g_ps = ps_g.tile([128, 128], F32, tag="G", name=f"G_{b}")
                      cs_ps = ps_g.tile([128, 1], F32, tag="CS", name=f"CS_{b}")
                      for t in range(NT8):
                          nc.tensor.matmul(g_ps[:], x8t[b][:, t, :, :],
                                           x8t[b][:, t, :, :],
                                           start=(t == 0), stop=(t == NT8 - 1),
                                           perf_mode=DR)
                          nc.tensor.matmul(cs_ps[:], x8t[b][:, t, :, :],
                                           ones8[:],
                                           start=(t == 0), stop=(t == NT8 - 1),
                                           perf_mode=DR)
                      # ---- project Gram -> (sum, sumsq) of pre1 ----
                      g_sbuf = w1p.tile([128, 128], F32R, tag="gsb", name=f"gsb_{b}")
                      nc.vector.tensor_copy(g_sbuf[:], g_ps[:])
                      mm1 = ps_pj.tile([128, 512], F32, tag="pj", name=f"mm1_{b}")
                      nc.tensor.matmul(mm1[:], g_sbuf[:], w1_r[b][:], start=True,
                                       stop=True)
                      prod = w1p.tile([128, 512], F32R, tag="prod", name=f"prod_{b}")
                      nc.vector.tensor_mul(prod[:], w1_r[b][:], mm1[:])
                      xsum_r = w1p.tile([128, 4], F32R, tag="xsumr",
                                        name=f"xsumr_{b}")
                      for q in range(4):
                          nc.vector.tensor_copy(xsum_r[:, q:q + 1], cs_ps[:])
                      for m in range(4):
                          sq = ps_pj.tile([128, 4], F32, tag="pj2", name=f"sq_{b}_{m}")
                          nc.tensor.matmul(sq[:], prod[:, ts(m, 128)], ones_r[:],
                                           start=True, stop=True)
                          nc.vector.tensor_copy(pay[0][b][:, m, 1:2], sq[:, 0:1])
                          sm = ps_pj.tile([128, 4], F32, tag="pj2", name=f"sm_{b}_{m}")
                          nc.tensor.matmul(sm[:], w1_r[b][:, ts(m, 128)], xsum_r[:],
                                           start=True, stop=True)
                          nc.vector.tensor_copy(pay[0][b][:, m, 0:1], sm[:, 0:1])
                      issue_allreduce(0, b)
                      load_pass2_weights(b, pay_dma[(0, b)].ins)
                      if b == 0:
                          # bulk inputs, gated behind the AR(0,0) payload DMA.
                          # xT(b0,s0) first (pass-2 chunk 0 needs it), then
                          # x8_1 (gram_1 -> AR(0,1)), then the rest.
                          gate = pay_dma[(0, 0)].ins

                          def xt_dma(b2, s):
                              return nc.sync.dma_start(
                                  xT[:, b2, s * XSL:(s + 1) * XSL],
                                  xTd[:, b2, s * XSL:(s + 1) * XSL])

                          bulk = [xt_dma(0, 0), x8_dma(1, 0, NH),
                                  x8_dma(1, NH, NT8)]
                          bulk += [xt_dma(0, s) for s in range(1, XS)]
                          bulk += [xt_dma(1, s) for s in range(XS)]
                          for d in bulk:
                              _add_dep_helper(d.ins, gate, sync=True,
                                              reason="bulk after AR payload")
                          bulk_gate = bulk[-1].ins

              # ================= pass 2: L1 -> BN1 -> L2 -> stats/spill ====
              with (
                  tc.tile_pool(name="w2p", bufs=4) as w2p,
                  tc.tile_pool(name="ps_p1", bufs=4, space="PSUM") as ps_p1,
                  tc.tile_pool(name="ps_p2", bufs=3, space="PSUM") as ps_p2,
              ):
                  for b in range(2):
                      finish_stats(0, b)
                      for c, (c0, tc_sz) in enumerate(CHUNKS):
                          if b == 1 and c == 6:
                              # AR(1,0) is long done by now; computing its
                              # scale/shift here keeps pass-3 startup off the
                              # critical path
                              finish_stats(1, 0)
                          h1 = w2p.tile([128, 4, tc_sz], BF16, tag="h1",
                                        name=f"h1_{c}_{b}")
                          for m in range(4):
                              pp = ps_p1.tile([128, tc_sz], F32, tag="p1",
                                              name=f"p1_{c}_{b}_{m}")
                              nc.tensor.matmul(pp[:], w1_bf[b][:, ts(m, 128)],
                                               xT[:, b, c0:c0 + tc_sz],
                                               start=True, stop=True)
                              nc.scalar.activation(
                                  h1[:, m, :], pp[:], AF.Relu,
                                  bias=shift_t[0][:, b, m:m + 1],
                                  scale=scale_t[0][:, b, m:m + 1])
                          if c == C - 1:
                              # padded nodes: relu(shift) != 0 would pollute BN2 stats
                              nc.scalar.mul(h1[:, :, PAD0:], h1[:, :, PAD0:], 0.0)
                          spl = w2p.tile([128, 4, tc_sz], BF16, tag="spl",
                                         name=f"spl_{c}_{b}")
                          for m in range(4):
                              pq = ps_p2.tile([128, tc_sz], F32, tag="p2",
                                              name=f"p2_{c}_{b}_{m}")
                              for k in range(4):
                                  nc.tensor.matmul(pq[:], w2_t[b][:, k, ts(m, 128)],
                                                   h1[:, k, :],
                                                   start=(k == 0), stop=(k == 3))
                              # copy frees the PSUM bank; stats read the SBUF
                              # copy and can lag without stalling the PE.
                              # Last chunks go all-DVE so ACT is free to start
                              # pass-3's h2 immediately.
                              if m % 2 == 0 and not (b == 1 and c >= C - 2):
                                  nc.scalar.copy(spl[:, m, :], pq[:])
                              else:
                                  nc.vector.tensor_copy(spl[:, m, :], pq[:])
                          for m in range(4):
                              nc.vector.bn_stats(st2[:, b, m, c, :], spl[:, m, :])
                          nc.sync.dma_start(spill[:, b, :, c0:c0 + tc_sz], spl[:])
                      # ---- aggregate local BN2 stats, launch AllReduce ----
                      agg = stat.tile([128, 4, 2], F32, tag="agg", name=f"agg_{b}")
                      for m in range(4):
                          nc.vector.bn_aggr(agg[:, m, :], st2[:, b, m, :, :])
                      tmp2 = stat.tile([128, 4], F32, tag="tmp2", name=f"tmp2_{b}")
                      nc.vector.tensor_scalar_mul(pay[1][b][:, :, 0], agg[:, :, 0],
                                                  float(NP))
                      nc.vector.tensor_mul(tmp2[:], agg[:, :, 0], agg[:, :, 0])
                      nc.vector.tensor_add(tmp2[:], tmp2[:], agg[:, :, 1])
                      nc.vector.tensor_scalar_mul(pay[1][b][:, :, 1], tmp2[:],
                                                  float(NP))
                      issue_allreduce(1, b)
                      if b == 0:
                          load_pass3_weights(bulk_gate)
                          # prefetch first b0 spill chunks on the gpsimd
                          # SWDGE queue (drains right after AR(1,0)), so
                          # pass-3 b0 sections can run during AR(1,1)
                          for cq in range(PF3):
                              cq0, cqs = CHUNKS[cq]
                              pftile = pf3.tile([128, 4, cqs], BF16, tag="pf",
                                                name=f"pf3_{cq}")
                              nc.gpsimd.dma_start(
                                  pftile[:], spill[:, 0, :, cq0:cq0 + cqs])
                              pf3_tiles[cq] = pftile
                          load_stats(1, 0, dma_engine=nc.gpsimd)

            # ================= pass 3: BN2 -> Lf -> fp8 head =============
            with (
                tc.tile_pool(name="w3p", bufs=2) as w3p,
                tc.tile_pool(name="ps_f", bufs=4, space="PSUM") as ps_f,
                tc.tile_pool(name="ps_t", bufs=4, space="PSUM") as ps_t,
            ):
                ps_o = ps_t  # share the t/o banks (tag-separated slots share pool)

                f_tiles = {}

                def b_section(c, b):
                    """BN2+relu -> Lf -> fp8 hi/lo split of f for branch b."""
                    c0, tc_sz = CHUNKS[c]
                    fhi = w3p.tile([128, 4, tc_sz], FP8, tag=f"fhi{b}",
                                   bufs=(PF3 + 1 if b == 0 else 2),
                                   name=f"fhi{b}_{c}")
                    flo = w3p.tile([128, 4, tc_sz], FP8, tag=f"flo{b}",
                                   bufs=(PF3 + 1 if b == 0 else 2),
                                   name=f"flo{b}_{c}")
                    if b == 0:
                        f_tiles[c] = (fhi, flo)
                    if b == 0 and c in pf3_tiles:
                        pre2 = pf3_tiles.pop(c)
                    else:
                        pre2 = w3p.tile([128, 4, tc_sz], BF16, tag="pre2ld",
                                        bufs=4, name=f"pre2_{c}_{b}")
                        nc.sync.dma_start(pre2[:], spill[:, b, :, c0:c0 + tc_sz])
                    h2 = w3p.tile([128, 4, tc_sz], BF16, tag="h2", bufs=3,
                                  name=f"h2_{c}_{b}")
                    for k in range(4):
                        nc.scalar.activation(
                            h2[:, k, :], pre2[:, k, :], AF.Relu,
                            bias=shift_t[1][:, b, k:k + 1],
                            scale=scale_t[1][:, b, k:k + 1])
                    for m in range(4):
                        pf = ps_f.tile([128, tc_sz], F32, tag="f",
                                       name=f"pf_{c}_{b}_{m}")
                        for k in range(4):
                            nc.tensor.matmul(pf[:], wf_t[b][:, k, ts(m, 128)],
                                             h2[:, k, :],
                                             start=(k == 0), stop=(k == 3))
                        # split hi-copies between ACT and DVE so neither
                        # engine paces the (DVE-heavy) f-split
                        if b == 0 and m % 2 == 0:
                            nc.scalar.copy(fhi[:, m, :], pf[:])
                        else:
                            nc.vector.tensor_copy(fhi[:, m, :], pf[:])
                        nc.vector.scalar_tensor_tensor(
                            flo[:, m, :], pf[:], 1.0, fhi[:, m, :],
                            op0=ALU.mult, op1=ALU.subtract)
                    return fhi, flo

                # run-ahead: b0 sections of the prefetched chunks execute
                # while AllReduce (1,1) is still in flight
                for c in range(PF3):
                    b_section(c, 0)
                    if c == 5:
                        finish_stats(1, 1)

                for c, (c0, tc_sz) in enumerate(CHUNKS):
                    if c >= PF3:
                        b_section(c, 0)
                    f0hi, f0lo = f_tiles.pop(c)
                    f1hi, f1lo = b_section(c, 1)
                    # ---- head: Wh1 fp8 DoubleRow 3-term ----
                    t_sb = w3p.tile([128, 4, tc_sz], BF16, tag="t_sb", bufs=2,
                                    name=f"t_sb_{c}")
                    for m in range(4):
                        ptl = ps_t.tile([128, tc_sz], F32, tag="t",
                                        name=f"ptl_{c}_{m}")
                        n_mm = 0
                        for fh, fl, kb in ((f0hi, f0lo, 0), (f1hi, f1lo, 4)):
                            for j in range(2):
                                wsl_h = wh1h[:, kb + 2 * j:kb + 2 * j + 2, ts(m, 128)]
                                wsl_l = wh1l[:, kb + 2 * j:kb + 2 * j + 2, ts(m, 128)]
                                fsl_h = fh[:, 2 * j:2 * j + 2, :]
                                fsl_l = fl[:, 2 * j:2 * j + 2, :]
                                for wsl, fsl in ((wsl_h, fsl_h), (wsl_h, fsl_l),
                                                 (wsl_l, fsl_h)):
                                    nc.tensor.matmul(ptl[:], wsl, fsl,
                                                     start=(n_mm == 0),
                                                     stop=(n_mm == 11),
                                                     perf_mode=DR)
                                    n_mm += 1
                        nc.scalar.activation(t_sb[:, m, :], ptl[:], AF.Relu,
                                             bias=bh1_sb[:, m:m + 1],
                                             scale=1.0 / 64.0)
                    po = ps_o.tile([10, tc_sz], F32, tag="t", name=f"po_{c}")
                    for k in range(4):
                        nc.tensor.matmul(po[:], wh2_t[:, k, :], t_sb[:, k, :],
                                         start=(k == 0), stop=(k == 3))
                    o_sb = w3p.tile([10, tc_sz], F32, tag="o_sb", name=f"o_sb_{c}")
                    nc.scalar.activation(o_sb[:], po[:], AF.Identity,
                                         bias=bh2_sb[:, 0:1])
                    nc.sync.dma_start(outd[:, c0:c0 + tc_sz], o_sb[:])

    nc.compile()
    return nc


def _get_program():
    if "nc" not in _CACHE:
        _CACHE["nc"] = _build_program()
    return _CACHE["nc"]


def kernel(**inputs):
    nc = _get_program()
    F8 = ml_dtypes.float8_e4m3
    B16 = ml_dtypes.bfloat16

    def shard_pad(x):
        x = np.ascontiguousarray(x, dtype=np.float32).reshape(NCORES, NSH, 128)
        pad = np.zeros((NCORES, NP - NSH, 128), dtype=np.float32)
        return np.concatenate([x, pad], axis=1)  # [NCORES, NP, 128]

    xp = [shard_pad(inputs["x_1"]), shard_pad(inputs["x_2"])]
    # xT: [NCORES, 128, 2, NP] bf16
    xT = np.stack([np.swapaxes(xp[0], 1, 2), np.swapaxes(xp[1], 1, 2)],
                  axis=1).astype(B16)  # [NCORES, 2, 128, NP]
    xT = np.ascontiguousarray(np.swapaxes(xT, 1, 2))  # [NCORES, 128, 2, NP]
    # xin8: [NCORES, 128, NT8, 256] fp8 per branch (node-within-tile on the
    # partition axis, contiguous per partition row for fast DMA)
    x8 = [np.ascontiguousarray(
              xp[b].reshape(NCORES, NT8, 2, 128, 128).transpose(0, 3, 1, 2, 4)
              .reshape(NCORES, 128, NT8, 256)).astype(F8)
          for b in range(2)]

    rep = {}
    # W1 stacked [128, 2, 512]
    rep["W1S"] = np.ascontiguousarray(
        np.stack([inputs["W1_1"], inputs["W1_2"]], axis=1), dtype=np.float32)
    for nm in ("W2_1", "W2_2", "Wf_1", "Wf_2"):
        rep[nm] = np.ascontiguousarray(inputs[nm]).astype(B16)
    rep["Wh2"] = np.ascontiguousarray(inputs["Wh2"]).astype(B16)
    for nm in ("g1_1", "be1_1", "g2_1", "be2_1",
               "g1_2", "be1_2", "g2_2", "be2_2", "bh2"):
        rep[nm] = np.ascontiguousarray(inputs[nm], dtype=np.float32)

    # Wh1 -> x64 fp8 hi/lo pair in [128, 8, 512] (p k m) layout
    wh1 = np.ascontiguousarray(inputs["Wh1"], dtype=np.float32) * 64.0
    wh1_pkm = wh1.reshape(8, 128, 512).swapaxes(0, 1)  # [128, 8, 512]
    whi = wh1_pkm.astype(F8)
    wlo = (wh1_pkm - whi.astype(np.float32)).astype(F8)
    rep["WH1HI"] = np.ascontiguousarray(whi)
    rep["WH1LO"] = np.ascontiguousarray(wlo)
    # bh1' = bh1 + Wh1^T [bf_1; bf_2]  (absorbs the Lf biases)
    bfcat = np.concatenate([np.asarray(inputs["bf_1"], np.float64),
                            np.asarray(inputs["bf_2"], np.float64)])
    rep["BH1P"] = (np.asarray(inputs["bh1"], np.float64)
                   + bfcat @ np.asarray(inputs["Wh1"], np.float64)).astype(np.float32)

    rep["AUX"] = np.ones((128, 4), dtype=np.float32)
    rep["EPSA"] = np.full((128, 1), EPS, dtype=np.float32)

    in_maps = []
    for c in range(NCORES):
        m = {"XT": xT[c], "XIN8_1": x8[0][c], "XIN8_2": x8[1][c]}
        m.update(rep)
        in_maps.append(m)

    res = bass_utils.run_bass_kernel_spmd(nc, in_maps, core_ids=list(range(NCORES)))
    parts = [res.results[c]["OUT"][:, :NSH] for c in range(NCORES)]
    out = np.concatenate(parts, axis=1).T
    return np.ascontiguousarray(out, dtype=np.float32)


# revision 35
# speedup vs baseline: 1.0226x; 1.0202x over previous
"""ChebyNet (K=1) dual-branch MLP + BN kernel for 8 Trainium2 NeuronCores.

Network (per reference):
  branch b in {1,2}:  h = relu(BN(x_b @ W1_b)) ; h = relu(BN(h @ W2_b)) ; f_b = h @ Wf_b + bf_b
  out = relu(concat(f_1, f_2) @ Wh1 + bh1) @ Wh2 + bh2

ChebConv with K=1 ignores edge_index/edge_weight entirely.  Training-mode
BatchNorm over the node axis makes the linear-layer biases b1/b2 cancel
exactly, so they are never loaded.  bf_b is absorbed into bh1 on the host
(bh1' = bh1 + Wh1^T [bf_1; bf_2]), so the Lf output f is bias-free.

Sharding: nodes (axis 0) split across 8 cores, 12500 each, zero-padded to
12544 = 98*128.  Weights replicated.  BN batch stats are combined with an
AllReduce(add) of per-core (sum, sumsq) over the 8 cores; one collective
per (layer, branch), interleaved so each hides under surrounding compute.

Layouts/dtypes:
 - x arrives host-transposed as bf16 xT [feat, branch, node] (no PE
   transposes) plus a node-major fp8 copy xin8 used only for the layer-1
   Gram-matrix BN statistics (X^T X), computed with fp8 DoubleRow matmuls.
 - L1 runs in bf16; L2/Lf/Wh2 in bf16; the big head GEMM Wh1 (K=1024) runs
   in fp8e4m3 DoubleRow with 3-term error compensation:
      Wh1*64 = Whi + Wlo (host-split fp8 pair), f = fhi + flo (device split)
      t = (Whi.fhi + Whi.flo + Wlo.fhi)/64  -- residual ~1e-3 relative.
   fhi = fp8(f) on DVE (copy from PSUM), flo = fp8(psum - fhi) in a single
   fused scalar_tensor_tensor, replacing the old bias-add pass (bias is
   absorbed into bh1').  DoubleRow packs 2 k-tiles per matmul at 0.5
   cycles/row, so Wh1 costs 0.375x its bf16 cycles.
 - Layer-1 BN stats use the Gram identity: sumsq(pre1) = diag(W1^T (X^T X) W1)
   and sum(pre1) = W1^T (X^T 1), so pre1 is never materialized in pass 1.
"""

import os

os.environ.setdefault("JAX_PLATFORMS", "axon,cpu")

import numpy as np
import ml_dtypes

import concourse.bacc as bacc
import concourse.mybir as mybir
import concourse.tile as tile
from concourse import bass_utils
from concourse.bass import ts, _add_dep_helper

F32 = mybir.dt.float32
F32R = mybir.dt.float32r
BF16 = mybir.dt.bfloat16
FP8 = mybir.dt.float8e4
AF = mybir.ActivationFunctionType
ALU = mybir.AluOpType
DR = mybir.MatmulPerfMode.DoubleRow

NTOT = 100000          # true node count
NCORES = 8
NSH = NTOT // NCORES   # 12500 true nodes per core
NP = 12544             # padded per-core nodes (= 98 * 128)
NT8 = NP // 256        # 49 gram node-tile pairs
T = 512                # node-chunk size (free dim of matmuls / PSUM bank)
CHUNKS = [(i * T, T) for i in range(NP // T)] + ([(NP - NP % T, NP % T)] if NP % T else [])
C = len(CHUNKS)
XS = 4                 # xT DMA slices per branch (for xsum overlap)
PAD0 = NSH - (NP - (NP % T or T))  # first padded column inside last chunk (212)
EPS = 1e-5
PF3 = 8                # pass-3 b0 run-ahead chunks (hides AllReduce(1,1))

_CACHE = {}


def _build_program():
    nc = bacc.Bacc("TRN2", target_bir_lowering=False, debug=False,
                   num_devices=NCORES)

    # ---- kernel I/O -----------------------------------------------------
    xTd = nc.dram_tensor("XT", [128, 2, NP], BF16, kind="ExternalInput")
    x8d = [nc.dram_tensor(f"XIN8_{b+1}", [128, NT8, 256], FP8,
                          kind="ExternalInput") for b in range(2)]
    w1d = nc.dram_tensor("W1S", [128, 2, 512], F32, kind="ExternalInput")
    w2d = [nc.dram_tensor(f"W2_{b+1}", [512, 512], BF16, kind="ExternalInput")
           for b in range(2)]
    wfd = [nc.dram_tensor(f"Wf_{b+1}", [512, 512], BF16, kind="ExternalInput")
           for b in range(2)]
    gd = [[nc.dram_tensor(f"g{l+1}_{b+1}", [512], F32, kind="ExternalInput")
           for b in range(2)] for l in range(2)]
    bed = [[nc.dram_tensor(f"be{l+1}_{b+1}", [512], F32, kind="ExternalInput")
            for b in range(2)] for l in range(2)]
    whhd = nc.dram_tensor("WH1HI", [128, 8, 512], FP8, kind="ExternalInput")
    whld = nc.dram_tensor("WH1LO", [128, 8, 512], FP8, kind="ExternalInput")
    bh1d = nc.dram_tensor("BH1P", [512], F32, kind="ExternalInput")
    wh2d = nc.dram_tensor("Wh2", [512, 10], BF16, kind="ExternalInput")
    bh2d = nc.dram_tensor("bh2", [10], F32, kind="ExternalInput")
    auxd = nc.dram_tensor("AUX", [128, 4], F32R, kind="ExternalInput")
    epsd = nc.dram_tensor("EPSA", [128, 1], F32, kind="ExternalInput")
    outd = nc.dram_tensor("OUT", [10, NP], F32, kind="ExternalOutput")

    # ---- DRAM scratch ---------------------------------------------------
    spill = nc.dram_tensor("pre2_spill", [128, 2, 4, NP], BF16)
    cc_in = [[nc.dram_tensor(f"cc{l}{b}_in", [128, 4, 2], F32) for b in range(2)]
             for l in range(2)]
    cc_out = [[nc.dram_tensor(f"cc{l}{b}_out", [NCORES, 128, 4, 2], F32,
                              addr_space="Shared") for b in range(2)]
              for l in range(2)]

    def vec_ap(h, p=128):
        return h.ap().rearrange("(m p) -> p m", p=p)

    with tile.TileContext(nc) as tc:
        with (
            tc.tile_pool(name="wpool", bufs=1) as wp,
            tc.tile_pool(name="stat", bufs=1) as stat,
            tc.tile_pool(name="pf3", bufs=PF3) as pf3,
        ):
            pf3_tiles = {}
            ones_r = wp.tile([128, 4], F32R, name="ones_r")
            nc.sync.dma_start(ones_r[:], auxd[:, :])
            eps_t = stat.tile([128, 1], F32, name="eps_t")
            nc.scalar.dma_start(eps_t[:], epsd[:, :])
            # prime the ACT function tables during the input-DMA wait so the
            # first real Relu/Sqrt doesn't pay the 1.3us table load
            warm = stat.tile([128, 1], F32, name="warm")
            nc.scalar.activation(warm[:], eps_t[:], AF.Relu)
            nc.scalar.activation(warm[:], eps_t[:], AF.Sqrt, bias=eps_t[:])
            nc.scalar.activation(warm[:], eps_t[:], AF.Identity, bias=eps_t[:])

            # W1 now (pass-1 projection needs it); everything else deferred.
            w1f = wp.tile([128, 2, 512], F32, name="w1f")
            nc.scalar.dma_start(w1f[:], w1d[:, :, :])
            w1_bf, w1_r = [], []
            for b in range(2):
                w1b = wp.tile([128, 512], BF16, name=f"w1b_{b}")
                nc.vector.tensor_copy(w1b[:], w1f[:, b, :])
                w1r = wp.tile([128, 512], F32R, name=f"w1r_{b}")
                nc.vector.tensor_copy(w1r[:], w1b[:])
                w1_bf.append(w1b)
                w1_r.append(w1r)

            # tiles declared up front, DMAs emitted later via the loaders
            w2_t = [wp.tile([128, 4, 512], BF16, name=f"w2_{b}") for b in range(2)]
            wf_t = [wp.tile([128, 4, 512], BF16, name=f"wf_{b}") for b in range(2)]
            wh1h = wp.tile([128, 8, 512], FP8, name="wh1h")
            wh1l = wp.tile([128, 8, 512], FP8, name="wh1l")
            wh2_t = wp.tile([128, 4, 10], BF16, name="wh2_t")
            bh1_sb = wp.tile([128, 4], F32, name="bh1_sb")
            bh2_sb = wp.tile([10, 1], F32, name="bh2_sb")
            g_sb = [stat.tile([128, 2, 4], F32, name=f"g_sb{l}") for l in range(2)]
            be_sb = [stat.tile([128, 2, 4], F32, name=f"be_sb{l}") for l in range(2)]

            def load_pass2_weights(b):
                nc.scalar.dma_start(
                    w2_t[b][:], w2d[b].ap().rearrange("(k p) m -> p k m", p=128))
                nc.scalar.dma_start(g_sb[0][:, b, :], vec_ap(gd[0][b]))
                nc.scalar.dma_start(be_sb[0][:, b, :], vec_ap(bed[0][b]))

            def load_pass3_weights():
                for b in range(2):
                    nc.scalar.dma_start(
                        wf_t[b][:], wfd[b].ap().rearrange("(k p) m -> p k m", p=128))
                    nc.scalar.dma_start(g_sb[1][:, b, :], vec_ap(gd[1][b]))
                    nc.scalar.dma_start(be_sb[1][:, b, :], vec_ap(bed[1][b]))
                nc.scalar.dma_start(wh1h[:], whhd[:, :, :])
                nc.scalar.dma_start(wh1l[:], whld[:, :, :])
                nc.scalar.dma_start(
                    wh2_t[:], wh2d.ap().rearrange("(k p) m -> p k m", p=128))
                nc.scalar.dma_start(bh1_sb[:], vec_ap(bh1d))
                nc.scalar.dma_start(
                    bh2_sb[:], bh2d.ap().rearrange("(m o) -> m o", o=1))

            st2 = stat.tile([128, 2, 4, C, 6], F32, name="st2")
            pay = [[stat.tile([128, 4, 2], F32, name=f"pay{l}{b}")
                    for b in range(2)] for l in range(2)]
            scale_t = [stat.tile([128, 2, 4], F32, name=f"scale{l}") for l in range(2)]
            shift_t = [stat.tile([128, 2, 4], F32, name=f"shift{l}") for l in range(2)]

            pay_dma = {}

            def issue_allreduce(l, b):
                # payload on the SWDGE queue: never queues behind bulk input
                # or spill DMAs, so the collective launches immediately.
                # AllGather + local sum is ~2x faster than AllReduce.
                pay_dma[(l, b)] = nc.gpsimd.dma_start(
                    cc_in[l][b][:, :, :], pay[l][b][:])
                nc.gpsimd.collective_compute(
                    "AllGather", mybir.AluOpType.bypass,
                    replica_groups=[list(range(NCORES))],
                    ins=[cc_in[l][b].ap().opt()], outs=[cc_out[l][b].ap().opt()],
                )

            gl_tiles = {}

            def load_stats(l, b, dma_engine=None):
                gl = stat.tile([128, NCORES, 4, 2], F32, tag=f"gl{l}{b}",
                               name=f"gl{l}{b}")
                (dma_engine or nc.sync).dma_start(
                    gl[:], cc_out[l][b].ap().rearrange("c p m s -> p c m s"))
                gl_tiles[(l, b)] = gl

            def finish_stats(l, b):
                """cc_out[l][b] -> scale_t[l][:, b, :], shift_t[l][:, b, :]."""
                if (l, b) not in gl_tiles:
                    load_stats(l, b)
                glg = gl_tiles.pop((l, b))
                # sum the 8 gathered per-core payloads (3-level tree)
                s4 = stat.tile([128, 4, 4, 2], F32, tag="s4", name=f"s4_{l}{b}")
                nc.vector.tensor_add(s4[:], glg[:, 0:4, :, :], glg[:, 4:8, :, :])
                s2 = stat.tile([128, 2, 4, 2], F32, tag="s2", name=f"s2_{l}{b}")
                nc.vector.tensor_add(s2[:], s4[:, 0:2, :, :], s4[:, 2:4, :, :])
                gl = stat.tile([128, 4, 2], F32, tag=f"gls{l}{b}",
                               name=f"gls{l}{b}")
                nc.vector.tensor_add(gl[:], s2[:, 0, :, :], s2[:, 1, :, :])
                mu = stat.tile([128, 4], F32, tag="mu", name=f"mu{l}{b}")
                var = stat.tile([128, 4], F32, tag="var", name=f"var{l}{b}")
                tmp = stat.tile([128, 4], F32, tag="tmpf", name=f"tmp{l}{b}")
                nc.vector.tensor_scalar_mul(mu[:], gl[:, :, 0], 1.0 / NTOT)
                nc.vector.tensor_scalar_mul(var[:], gl[:, :, 1], 1.0 / NTOT)
                nc.vector.tensor_mul(tmp[:], mu[:], mu[:])
                nc.vector.tensor_sub(var[:], var[:], tmp[:])
                nc.scalar.activation(var[:], var[:], AF.Sqrt, bias=eps_t[:])
                nc.vector.reciprocal(var[:], var[:])
                nc.vector.tensor_mul(scale_t[l][:, b, :], g_sb[l][:, b, :], var[:])
                nc.vector.tensor_mul(tmp[:], mu[:], scale_t[l][:, b, :])
                nc.vector.tensor_sub(shift_t[l][:, b, :], be_sb[l][:, b, :], tmp[:])

            # ================= passes 1+2 (share the resident xT) ========
            with tc.tile_pool(name="xtp", bufs=1) as xtp:
              # resident transposed input, bf16: [feat, branch, node]
              xT = xtp.tile([128, 2, NP], BF16, name="xT")

              # ---- pass 1: DMA + fp8 Gram stats ----
              with (
                tc.tile_pool(name="w1p", bufs=2) as w1p,
                tc.tile_pool(name="ps_g", bufs=1, space="PSUM") as ps_g,
                tc.tile_pool(name="ps_pj", bufs=1, space="PSUM") as ps_pj,
              ):
                  XSL = NP // XS
                  ones8 = wp.tile([128, 2, 1], FP8, name="ones8")
                  nc.vector.tensor_copy(ones8[:], ones_r[:, 0:2].rearrange(
                      "p (s o) -> p s o", o=1))
                  NH = NT8 // 2

                  def x8_dma(b, lo, hi):
                      return nc.sync.dma_start(
                          x8t[b][:, lo:hi, :, :],
                          x8d[b].ap()[:, lo:hi, :].rearrange(
                              "p t (s f) -> p t s f", s=2))

                  x8t = [w1p.tile([128, NT8, 2, 128], FP8, name=f"x8_{b}")
                         for b in range(2)]

                  def xt_dma(b2, s):
                      return nc.sync.dma_start(
                          xT[:, b2, s * XSL:(s + 1) * XSL],
                          xTd[:, b2, s * XSL:(s + 1) * XSL])

                  x8_dma(0, 0, NH)
                  x8_dma(0, NH, NT8)
                  for s in range(XS):
                      xt_dma(0, s)
                  x8_dma(1, 0, NH)
                  x8_dma(1, NH, NT8)
                  for s in range(XS):
                      xt_dma(1, s)
                  for b in range(2):
                      # Gram X^T X and colsum X^T 1, both via fp8 DoubleRow
                      g_ps = ps_g.tile([128, 128], F32, tag="G", name=f"G_{b}")
                      cs_ps = ps_g.tile([128, 1], F32, tag="CS", name=f"CS_{b}")
                      for t in range(NT8):
                          nc.tensor.matmul(g_ps[:], x8t[b][:, t, :, :],
                                           x8t[b][:, t, :, :],
                                           start=(t == 0), stop=(t == NT8 - 1),
                                           perf_mode=DR)
                          nc.tensor.matmul(cs_ps[:], x8t[b][:, t, :, :],
                                           ones8[:],
                                           start=(t == 0), stop=(t == NT8 - 1),
                                           perf_mode=DR)
                      # ---- project Gram -> (sum, sumsq) of pre1 ----
                      g_sbuf = w1p.tile([128, 128], F32R, tag="gsb", name=f"gsb_{b}")
                      nc.vector.tensor_copy(g_sbuf[:], g_ps[:])
                      mm1 = ps_pj.tile([128, 512], F32, tag="pj", name=f"mm1_{b}")
                      nc.tensor.matmul(mm1[:], g_sbuf[:], w1_r[b][:], start=True,
                                       stop=True)
                      prod = w1p.tile([128, 512], F32R, tag="prod", name=f"prod_{b}")
                      nc.vector.tensor_mul(prod[:], w1_r[b][:], mm1[:])
                      xsum_r = w1p.tile([128, 4], F32R, tag="xsumr",
                                        name=f"xsumr_{b}")
                      for q in range(4):
                          nc.vector.tensor_copy(xsum_r[:, q:q + 1], cs_ps[:])
                      sq = ps_pj.tile([128, 4, 4], F32, tag="sq", name=f"sq_{b}")
                      sm = ps_pj.tile([128, 4, 4], F32, tag="sm", name=f"sm_{b}")
                      for m in range(4):
                          nc.tensor.matmul(sq[:, m, :], prod[:, ts(m, 128)],
                                           ones_r[:], start=(m == 0),
                                           stop=(m == 3), skip_group_check=True)
                      for m in range(4):
                          nc.tensor.matmul(sm[:, m, :], w1_r[b][:, ts(m, 128)],
                                           xsum_r[:], start=(m == 0),
                                           stop=(m == 3), skip_group_check=True)
                      nc.vector.tensor_copy(pay[0][b][:, :, 1:2], sq[:, :, 0:1])
                      nc.vector.tensor_copy(pay[0][b][:, :, 0:1], sm[:, :, 0:1])
                      issue_allreduce(0, b)
                      load_pass2_weights(b)

              # ================= pass 2: L1 -> BN1 -> L2 -> stats/spill ====
              with (
                  tc.tile_pool(name="w2p", bufs=4) as w2p,
                  tc.tile_pool(name="ps_p1", bufs=4, space="PSUM") as ps_p1,
                  tc.tile_pool(name="ps_p2", bufs=3, space="PSUM") as ps_p2,
              ):
                  for b in range(2):
                      finish_stats(0, b)
                      for c, (c0, tc_sz) in enumerate(CHUNKS):
                          if b == 1 and c == 6:
                              # AR(1,0) is long done by now; computing its
                              # scale/shift here keeps pass-3 startup off the
                              # critical path
                              finish_stats(1, 0)
                          h1 = w2p.tile([128, 4, tc_sz], BF16, tag="h1",
                                        name=f"h1_{c}_{b}")
                          for m in range(4):
                              pp = ps_p1.tile([128, tc_sz], F32, tag="p1",
                                              name=f"p1_{c}_{b}_{m}")
                              nc.tensor.matmul(pp[:], w1_bf[b][:, ts(m, 128)],
                                               xT[:, b, c0:c0 + tc_sz],
                                               start=True, stop=True)
                              nc.scalar.activation(
                                  h1[:, m, :], pp[:], AF.Relu,
                                  bias=shift_t[0][:, b, m:m + 1],
                                  scale=scale_t[0][:, b, m:m + 1])
                          if c == C - 1:
                              # padded nodes: relu(shift) != 0 would pollute BN2 stats
                              nc.scalar.mul(h1[:, :, PAD0:], h1[:, :, PAD0:], 0.0)
                          spl = w2p.tile([128, 4, tc_sz], BF16, tag="spl",
                                         name=f"spl_{c}_{b}")
                          for m in range(4):
                              pq = ps_p2.tile([128, tc_sz], F32, tag="p2",
                                              name=f"p2_{c}_{b}_{m}")
                              for k in range(4):
                                  nc.tensor.matmul(pq[:], w2_t[b][:, k, ts(m, 128)],
                                                   h1[:, k, :],
                                                   start=(k == 0), stop=(k == 3))
                              # copy frees the PSUM bank; stats read the SBUF
                              # copy and can lag without stalling the PE.
                              # Last chunks go all-DVE so ACT is free to start
                              # pass-3's h2 immediately.
                              if m % 2 == 0 and not (b == 1 and c >= C - 2):
                                  nc.scalar.copy(spl[:, m, :], pq[:])
                              else:
                                  nc.vector.tensor_copy(spl[:, m, :], pq[:])
                          for m in range(4):
                              nc.vector.bn_stats(st2[:, b, m, c, :], spl[:, m, :])
                          nc.sync.dma_start(spill[:, b, :, c0:c0 + tc_sz], spl[:])
                      # ---- aggregate local BN2 stats, launch AllReduce ----
                      agg = stat.tile([128, 4, 2], F32, tag="agg", name=f"agg_{b}")
                      for m in range(4):
                          nc.vector.bn_aggr(agg[:, m, :], st2[:, b, m, :, :])
                      tmp2 = stat.tile([128, 4], F32, tag="tmp2", name=f"tmp2_{b}")
                      nc.vector.tensor_scalar_mul(pay[1][b][:, :, 0], agg[:, :, 0],
                                                  float(NP))
                      nc.vector.tensor_mul(tmp2[:], agg[:, :, 0], agg[:, :, 0])
                      nc.vector.tensor_add(tmp2[:], tmp2[:], agg[:, :, 1])
                      nc.vector.tensor_scalar_mul(pay[1][b][:, :, 1], tmp2[:],
                                                  float(NP))
                      issue_allreduce(1, b)
                      if b == 0:
                          load_pass3_weights()
                          # prefetch first b0 spill chunks on the gpsimd
                          # SWDGE queue (drains right after AR(1,0)), so
                          # pass-3 b0 sections can run during AR(1,1)
                          for cq in range(PF3):
                              cq0, cqs = CHUNKS[cq]
                              pftile = pf3.tile([128, 4, cqs], BF16, tag="pf",
                                                name=f"pf3_{cq}")
                              nc.gpsimd.dma_start(
                                  pftile[:], spill[:, 0, :, cq0:cq0 + cqs])
                              pf3_tiles[cq] = pftile
                          load_stats(1, 0, dma_engine=nc.gpsimd)

            # ================= pass 3: BN2 -> Lf -> fp8 head =============
            with (
                tc.tile_pool(name="w3p", bufs=2) as w3p,
                tc.tile_pool(name="ps_f", bufs=4, space="PSUM") as ps_f,
                tc.tile_pool(name="ps_t", bufs=4, space="PSUM") as ps_t,
            ):
                ps_o = ps_t  # share the t/o banks (tag-separated slots share pool)

                f_tiles = {}

                def b_section(c, b):
                    """BN2+relu -> Lf -> fp8 hi/lo split of f for branch b."""
                    c0, tc_sz = CHUNKS[c]
                    fhi = w3p.tile([128, 4, tc_sz], FP8, tag=f"fhi{b}",
                                   bufs=(PF3 + 1 if b == 0 else 2),
                                   name=f"fhi{b}_{c}")
                    flo = w3p.tile([128, 4, tc_sz], FP8, tag=f"flo{b}",
                                   bufs=(PF3 + 1 if b == 0 else 2),
                                   name=f"flo{b}_{c}")
                    if b == 0:
                        f_tiles[c] = (fhi, flo)
                    if b == 0 and c in pf3_tiles:
                        pre2 = pf3_tiles.pop(c)
                    else:
                        pre2 = w3p.tile([128, 4, tc_sz], BF16, tag="pre2ld",
                                        bufs=4, name=f"pre2_{c}_{b}")
                        nc.sync.dma_start(pre2[:], spill[:, b, :, c0:c0 + tc_sz])
                    h2 = w3p.tile([128, 4, tc_sz], BF16, tag="h2", bufs=3,
                                  name=f"h2_{c}_{b}")
                    for k in range(4):
                        nc.scalar.activation(
                            h2[:, k, :], pre2[:, k, :], AF.Relu,
                            bias=shift_t[1][:, b, k:k + 1],
                            scale=scale_t[1][:, b, k:k + 1])
                    for m in range(4):
                        pf = ps_f.tile([128, tc_sz], F32, tag="f",
                                       name=f"pf_{c}_{b}_{m}")
                        for k in range(4):
                            nc.tensor.matmul(pf[:], wf_t[b][:, k, ts(m, 128)],
                                             h2[:, k, :],
                                             start=(k == 0), stop=(k == 3))
                        # split hi-copies between ACT and DVE so neither
                        # engine paces the (DVE-heavy) f-split
                        if b == 0 and m % 2 == 0:
                            nc.scalar.copy(fhi[:, m, :], pf[:])
                        else:
                            nc.vector.tensor_copy(fhi[:, m, :], pf[:])
                        nc.vector.scalar_tensor_tensor(
                            flo[:, m, :], pf[:], 1.0, fhi[:, m, :],
                            op0=ALU.mult, op1=ALU.subtract)
                    return fhi, flo

                # run-ahead: b0 sections of the prefetched chunks execute
                # while AllReduce (1,1) is still in flight
                for c in range(PF3):
                    b_section(c, 0)
                    if c == 5:
                        finish_stats(1, 1)

                for c, (c0, tc_sz) in enumerate(CHUNKS):
                    if c >= PF3:
                        b_section(c, 0)
                    f0hi, f0lo = f_tiles.pop(c)
                    f1hi, f1lo = b_section(c, 1)
                    # ---- head: Wh1 fp8 DoubleRow 3-term ----
                    t_sb = w3p.tile([128, 4, tc_sz], BF16, tag="t_sb", bufs=2,
                                    name=f"t_sb_{c}")
                    for m in range(4):
                        ptl = ps_t.tile([128, tc_sz], F32, tag="t",
                                        name=f"ptl_{c}_{m}")
                        n_mm = 0
                        for fh, fl, kb in ((f0hi, f0lo, 0), (f1hi, f1lo, 4)):
                            for j in range(2):
                                wsl_h = wh1h[:, kb + 2 * j:kb + 2 * j + 2, ts(m, 128)]
                                wsl_l = wh1l[:, kb + 2 * j:kb + 2 * j + 2, ts(m, 128)]
                                fsl_h = fh[:, 2 * j:2 * j + 2, :]
                                fsl_l = fl[:, 2 * j:2 * j + 2, :]
                                for wsl, fsl in ((wsl_h, fsl_h), (wsl_h, fsl_l),
                                                 (wsl_l, fsl_h)):
                                    nc.tensor.matmul(ptl[:], wsl, fsl,
                                                     start=(n_mm == 0),
                                                     stop=(n_mm == 11),
                                                     perf_mode=DR)
                                    n_mm += 1
                        nc.scalar.activation(t_sb[:, m, :], ptl[:], AF.Relu,
                                             bias=bh1_sb[:, m:m + 1],
                                             scale=1.0 / 64.0)
                    po = ps_o.tile([10, tc_sz], F32, tag="t", name=f"po_{c}")
                    for k in range(4):
                        nc.tensor.matmul(po[:], wh2_t[:, k, :], t_sb[:, k, :],
                                         start=(k == 0), stop=(k == 3))
                    o_sb = w3p.tile([10, tc_sz], F32, tag="o_sb", name=f"o_sb_{c}")
                    nc.scalar.activation(o_sb[:], po[:], AF.Identity,
                                         bias=bh2_sb[:, 0:1])
                    nc.sync.dma_start(outd[:, c0:c0 + tc_sz], o_sb[:])

    nc.compile()
    return nc


def _get_program():
    if "nc" not in _CACHE:
        _CACHE["nc"] = _build_program()
    return _CACHE["nc"]


def kernel(**inputs):
    nc = _get_program()
    F8 = ml_dtypes.float8_e4m3
    B16 = ml_dtypes.bfloat16

    def shard_pad(x):
        x = np.ascontiguousarray(x, dtype=np.float32).reshape(NCORES, NSH, 128)
        pad = np.zeros((NCORES, NP - NSH, 128), dtype=np.float32)
        return np.concatenate([x, pad], axis=1)  # [NCORES, NP, 128]

    xp = [shard_pad(inputs["x_1"]), shard_pad(inputs["x_2"])]
    # xT: [NCORES, 128, 2, NP] bf16
    xT = np.stack([np.swapaxes(xp[0], 1, 2), np.swapaxes(xp[1], 1, 2)],
                  axis=1).astype(B16)  # [NCORES, 2, 128, NP]
    xT = np.ascontiguousarray(np.swapaxes(xT, 1, 2))  # [NCORES, 128, 2, NP]
    # xin8: [NCORES, 128, NT8, 256] fp8 per branch (node-within-tile on the
    # partition axis, contiguous per partition row for fast DMA)
    x8 = [np.ascontiguousarray(
              xp[b].reshape(NCORES, NT8, 2, 128, 128).transpose(0, 3, 1, 2, 4)
              .reshape(NCORES, 128, NT8, 256)).astype(F8)
          for b in range(2)]

    rep = {}
    # W1 stacked [128, 2, 512]
    rep["W1S"] = np.ascontiguousarray(
        np.stack([inputs["W1_1"], inputs["W1_2"]], axis=1), dtype=np.float32)
    for nm in ("W2_1", "W2_2", "Wf_1", "Wf_2"):
        rep[nm] = np.ascontiguousarray(inputs[nm]).astype(B16)
    rep["Wh2"] = np.ascontiguousarray(inputs["Wh2"]).astype(B16)
    for nm in ("g1_1", "be1_1", "g2_1", "be2_1",
               "g1_2", "be1_2", "g2_2", "be2_2", "bh2"):
        rep[nm] = np.ascontiguousarray(inputs[nm], dtype=np.float32)

    # Wh1 -> x64 fp8 hi/lo pair in [128, 8, 512] (p k m) layout
    wh1 = np.ascontiguousarray(inputs["Wh1"], dtype=np.float32) * 64.0
    wh1_pkm = wh1.reshape(8, 128, 512).swapaxes(0, 1)  # [128, 8, 512]
    whi = wh1_pkm.astype(F8)
    wlo = (wh1_pkm - whi.astype(np.float32)).astype(F8)
    rep["WH1HI"] = np.ascontiguousarray(whi)
    rep["WH1LO"] = np.ascontiguousarray(wlo)
    # bh1' = bh1 + Wh1^T [bf_1; bf_2]  (absorbs the Lf biases)
    bfcat = np.concatenate([np.asarray(inputs["bf_1"], np.float64),
                            np.asarray(inputs["bf_2"], np.float64)])
    rep["BH1P"] = (np.asarray(inputs["bh1"], np.float64)
                   + bfcat @ np.asarray(inputs["Wh1"], np.float64)).astype(np.float32)

    rep["AUX"] = np.ones((128, 4), dtype=np.float32)
    rep["EPSA"] = np.full((128, 1), EPS, dtype=np.float32)

    in_maps = []
    for c in range(NCORES):
        m = {"XT": xT[c], "XIN8_1": x8[0][c], "XIN8_2": x8[1][c]}
        m.update(rep)
        in_maps.append(m)

    res = bass_utils.run_bass_kernel_spmd(nc, in_maps, core_ids=list(range(NCORES)))
    parts = [res.results[c]["OUT"][:, :NSH] for c in range(NCORES)]
    out = np.concatenate(parts, axis=1).T
    return np.ascontiguousarray(out, dtype=np.float32)


# revision 37
# speedup vs baseline: 1.0433x; 1.0202x over previous
"""ChebyNet (K=1) dual-branch MLP + BN kernel for 8 Trainium2 NeuronCores.

Network (per reference):
  branch b in {1,2}:  h = relu(BN(x_b @ W1_b)) ; h = relu(BN(h @ W2_b)) ; f_b = h @ Wf_b + bf_b
  out = relu(concat(f_1, f_2) @ Wh1 + bh1) @ Wh2 + bh2

ChebConv with K=1 ignores edge_index/edge_weight entirely.  Training-mode
BatchNorm over the node axis makes the linear-layer biases b1/b2 cancel
exactly, so they are never loaded.  bf_b is absorbed into bh1 on the host
(bh1' = bh1 + Wh1^T [bf_1; bf_2]), so the Lf output f is bias-free.

Sharding: nodes (axis 0) split across 8 cores, 12500 each, zero-padded to
12544 = 98*128.  Weights replicated.  BN batch stats are combined with an
AllReduce(add) of per-core (sum, sumsq) over the 8 cores; one collective
per (layer, branch), interleaved so each hides under surrounding compute.

Layouts/dtypes:
 - x arrives host-transposed as bf16 xT [feat, branch, node] (no PE
   transposes) plus a node-major fp8 copy xin8 used only for the layer-1
   Gram-matrix BN statistics (X^T X), computed with fp8 DoubleRow matmuls.
 - L1 runs in bf16; L2/Lf/Wh2 in bf16; the big head GEMM Wh1 (K=1024) runs
   in fp8e4m3 DoubleRow with 3-term error compensation:
      Wh1*64 = Whi + Wlo (host-split fp8 pair), f = fhi + flo (device split)
      t = (Whi.fhi + Whi.flo + Wlo.fhi)/64  -- residual ~1e-3 relative.
   fhi = fp8(f) on DVE (copy from PSUM), flo = fp8(psum - fhi) in a single
   fused scalar_tensor_tensor, replacing the old bias-add pass (bias is
   absorbed into bh1').  DoubleRow packs 2 k-tiles per matmul at 0.5
   cycles/row, so Wh1 costs 0.375x its bf16 cycles.
 - Layer-1 BN stats use the Gram identity: sumsq(pre1) = diag(W1^T (X^T X) W1)
   and sum(pre1) = W1^T (X^T 1), so pre1 is never materialized in pass 1.
"""

import os

os.environ.setdefault("JAX_PLATFORMS", "axon,cpu")

import numpy as np
import ml_dtypes

import concourse.bacc as bacc
import concourse.mybir as mybir
import concourse.tile as tile
from concourse import bass_utils
from concourse.bass import ts, _add_dep_helper

F32 = mybir.dt.float32
F32R = mybir.dt.float32r
BF16 = mybir.dt.bfloat16
FP8 = mybir.dt.float8e4
AF = mybir.ActivationFunctionType
ALU = mybir.AluOpType
DR = mybir.MatmulPerfMode.DoubleRow

NTOT = 100000          # true node count
NCORES = 8
NSH = NTOT // NCORES   # 12500 true nodes per core
NP = 12544             # padded per-core nodes (= 98 * 128)
NT8 = NP // 256        # 49 gram node-tile pairs
T = 512                # node-chunk size (free dim of matmuls / PSUM bank)
CHUNKS = [(i * T, T) for i in range(NP // T)] + ([(NP - NP % T, NP % T)] if NP % T else [])
C = len(CHUNKS)
XS = 4                 # xT DMA slices per branch (for xsum overlap)
PAD0 = NSH - (NP - (NP % T or T))  # first padded column inside last chunk (212)
EPS = 1e-5
PF3 = 8                # pass-3 b0 run-ahead chunks (hides AllReduce(1,1))

_CACHE = {}


def _build_program():
    nc = bacc.Bacc("TRN2", target_bir_lowering=False, debug=False,
                   num_devices=NCORES)

    # ---- kernel I/O -----------------------------------------------------
    xTd = nc.dram_tensor("XT", [128, 2, NP], BF16, kind="ExternalInput")
    x8d = [nc.dram_tensor(f"XIN8_{b+1}", [128, NT8, 256], FP8,
                          kind="ExternalInput") for b in range(2)]
    w1d = nc.dram_tensor("W1S", [128, 2, 512], F32, kind="ExternalInput")
    w2d = [nc.dram_tensor(f"W2_{b+1}", [512, 512], BF16, kind="ExternalInput")
           for b in range(2)]
    wfd = [nc.dram_tensor(f"Wf_{b+1}", [512, 512], BF16, kind="ExternalInput")
           for b in range(2)]
    gd = [[nc.dram_tensor(f"g{l+1}_{b+1}", [512], F32, kind="ExternalInput")
           for b in range(2)] for l in range(2)]
    bed = [[nc.dram_tensor(f"be{l+1}_{b+1}", [512], F32, kind="ExternalInput")
            for b in range(2)] for l in range(2)]
    whhd = nc.dram_tensor("WH1HI", [128, 8, 512], FP8, kind="ExternalInput")
    whld = nc.dram_tensor("WH1LO", [128, 8, 512], FP8, kind="ExternalInput")
    bh1d = nc.dram_tensor("BH1P", [512], F32, kind="ExternalInput")
    wh2d = nc.dram_tensor("Wh2", [512, 10], BF16, kind="ExternalInput")
    bh2d = nc.dram_tensor("bh2", [10], F32, kind="ExternalInput")
    auxd = nc.dram_tensor("AUX", [128, 4], F32R, kind="ExternalInput")
    epsd = nc.dram_tensor("EPSA", [128, 1], F32, kind="ExternalInput")
    outd = nc.dram_tensor("OUT", [10, NP], F32, kind="ExternalOutput")

    # ---- DRAM scratch ---------------------------------------------------
    spill = nc.dram_tensor("pre2_spill", [128, 2, 4, NP], BF16)
    cc_in = [[nc.dram_tensor(f"cc{l}{b}_in", [128, 4, 2], F32) for b in range(2)]
             for l in range(2)]
    cc_out = [[nc.dram_tensor(f"cc{l}{b}_out", [NCORES, 128, 4, 2], F32,
                              addr_space="Shared") for b in range(2)]
              for l in range(2)]

    def vec_ap(h, p=128):
        return h.ap().rearrange("(m p) -> p m", p=p)

    with tile.TileContext(nc) as tc:
        with (
            tc.tile_pool(name="wpool", bufs=1) as wp,
            tc.tile_pool(name="stat", bufs=1) as stat,
            tc.tile_pool(name="pf3", bufs=PF3) as pf3,
        ):
            pf3_tiles = {}
            ones_r = wp.tile([128, 4], F32R, name="ones_r")
            nc.sync.dma_start(ones_r[:], auxd[:, :])
            eps_t = stat.tile([128, 1], F32, name="eps_t")
            nc.scalar.dma_start(eps_t[:], epsd[:, :])
            # prime the ACT function tables during the input-DMA wait so the
            # first real Relu/Sqrt doesn't pay the 1.3us table load
            warm = stat.tile([128, 1], F32, name="warm")
            nc.scalar.activation(warm[:], eps_t[:], AF.Relu)
            nc.scalar.activation(warm[:], eps_t[:], AF.Sqrt, bias=eps_t[:])
            nc.scalar.activation(warm[:], eps_t[:], AF.Identity, bias=eps_t[:])

            # W1 now (pass-1 projection needs it); everything else deferred.
            w1f = wp.tile([128, 2, 512], F32, name="w1f")
            nc.scalar.dma_start(w1f[:], w1d[:, :, :])
            w1_bf, w1_r = [], []
            for b in range(2):
                w1b = wp.tile([128, 512], BF16, name=f"w1b_{b}")
                nc.vector.tensor_copy(w1b[:], w1f[:, b, :])
                w1r = wp.tile([128, 512], F32R, name=f"w1r_{b}")
                nc.vector.tensor_copy(w1r[:], w1b[:])
                w1_bf.append(w1b)
                w1_r.append(w1r)

            # tiles declared up front, DMAs emitted later via the loaders
            w2_t = [wp.tile([128, 4, 512], BF16, name=f"w2_{b}") for b in range(2)]
            wf_t = [wp.tile([128, 4, 512], BF16, name=f"wf_{b}") for b in range(2)]
            wh1h = wp.tile([128, 8, 512], FP8, name="wh1h")
            wh1l = wp.tile([128, 8, 512], FP8, name="wh1l")
            wh2_t = wp.tile([128, 4, 10], BF16, name="wh2_t")
            bh1_sb = wp.tile([128, 4], F32, name="bh1_sb")
            bh2_sb = wp.tile([10, 1], F32, name="bh2_sb")
            g_sb = [stat.tile([128, 2, 4], F32, name=f"g_sb{l}") for l in range(2)]
            be_sb = [stat.tile([128, 2, 4], F32, name=f"be_sb{l}") for l in range(2)]

            def load_pass2_weights(b):
                nc.scalar.dma_start(
                    w2_t[b][:], w2d[b].ap().rearrange("(k p) m -> p k m", p=128))
                nc.scalar.dma_start(g_sb[0][:, b, :], vec_ap(gd[0][b]))
                nc.scalar.dma_start(be_sb[0][:, b, :], vec_ap(bed[0][b]))

            def load_pass3_weights():
                for b in range(2):
                    nc.scalar.dma_start(
                        wf_t[b][:], wfd[b].ap().rearrange("(k p) m -> p k m", p=128))
                    nc.scalar.dma_start(g_sb[1][:, b, :], vec_ap(gd[1][b]))
                    nc.scalar.dma_start(be_sb[1][:, b, :], vec_ap(bed[1][b]))
                nc.scalar.dma_start(wh1h[:], whhd[:, :, :])
                nc.scalar.dma_start(wh1l[:], whld[:, :, :])
                nc.scalar.dma_start(
                    wh2_t[:], wh2d.ap().rearrange("(k p) m -> p k m", p=128))
                nc.scalar.dma_start(bh1_sb[:], vec_ap(bh1d))
                nc.scalar.dma_start(
                    bh2_sb[:], bh2d.ap().rearrange("(m o) -> m o", o=1))

            st2 = stat.tile([128, 2, 4, C, 6], F32, name="st2")
            pay = [[stat.tile([128, 4, 2], F32, name=f"pay{l}{b}")
                    for b in range(2)] for l in range(2)]
            scale_t = [stat.tile([128, 2, 4], F32, name=f"scale{l}") for l in range(2)]
            shift_t = [stat.tile([128, 2, 4], F32, name=f"shift{l}") for l in range(2)]

            pay_dma = {}

            def issue_allreduce(l, b):
                # payload on the SWDGE queue: never queues behind bulk input
                # or spill DMAs, so the collective launches immediately.
                # AllGather + local sum is ~2x faster than AllReduce.
                pay_dma[(l, b)] = nc.gpsimd.dma_start(
                    cc_in[l][b][:, :, :], pay[l][b][:])
                nc.gpsimd.collective_compute(
                    "AllGather", mybir.AluOpType.bypass,
                    replica_groups=[list(range(NCORES))],
                    ins=[cc_in[l][b].ap().opt()], outs=[cc_out[l][b].ap().opt()],
                )

            gl_tiles = {}

            def load_stats(l, b, dma_engine=None):
                gl = stat.tile([128, NCORES, 4, 2], F32, tag=f"gl{l}{b}",
                               name=f"gl{l}{b}")
                (dma_engine or nc.sync).dma_start(
                    gl[:], cc_out[l][b].ap().rearrange("c p m s -> p c m s"))
                gl_tiles[(l, b)] = gl

            def finish_stats(l, b):
                """cc_out[l][b] -> scale_t[l][:, b, :], shift_t[l][:, b, :]."""
                if (l, b) not in gl_tiles:
                    load_stats(l, b)
                glg = gl_tiles.pop((l, b))
                # sum the 8 gathered per-core payloads (3-level tree)
                s4 = stat.tile([128, 4, 4, 2], F32, tag="s4", name=f"s4_{l}{b}")
                nc.vector.tensor_add(s4[:], glg[:, 0:4, :, :], glg[:, 4:8, :, :])
                s2 = stat.tile([128, 2, 4, 2], F32, tag="s2", name=f"s2_{l}{b}")
                nc.vector.tensor_add(s2[:], s4[:, 0:2, :, :], s4[:, 2:4, :, :])
                gl = stat.tile([128, 4, 2], F32, tag=f"gls{l}{b}",
                               name=f"gls{l}{b}")
                nc.vector.tensor_add(gl[:], s2[:, 0, :, :], s2[:, 1, :, :])
                mu = stat.tile([128, 4], F32, tag="mu", name=f"mu{l}{b}")
                var = stat.tile([128, 4], F32, tag="var", name=f"var{l}{b}")
                tmp = stat.tile([128, 4], F32, tag="tmpf", name=f"tmp{l}{b}")
                nc.vector.tensor_scalar_mul(mu[:], gl[:, :, 0], 1.0 / NTOT)
                nc.vector.tensor_scalar_mul(var[:], gl[:, :, 1], 1.0 / NTOT)
                nc.vector.tensor_mul(tmp[:], mu[:], mu[:])
                nc.vector.tensor_sub(var[:], var[:], tmp[:])
                nc.scalar.activation(var[:], var[:], AF.Sqrt, bias=eps_t[:])
                nc.vector.reciprocal(var[:], var[:])
                nc.vector.tensor_mul(scale_t[l][:, b, :], g_sb[l][:, b, :], var[:])
                nc.vector.tensor_mul(tmp[:], mu[:], scale_t[l][:, b, :])
                nc.vector.tensor_sub(shift_t[l][:, b, :], be_sb[l][:, b, :], tmp[:])

            # ================= passes 1+2 (share the resident xT) ========
            with tc.tile_pool(name="xtp", bufs=1) as xtp:
              # resident transposed input, bf16: [feat, branch, node]
              xT = xtp.tile([128, 2, NP], BF16, name="xT")

              # ---- pass 1: DMA + fp8 Gram stats ----
              with (
                tc.tile_pool(name="w1p", bufs=2) as w1p,
                tc.tile_pool(name="ps_g", bufs=1, space="PSUM") as ps_g,
                tc.tile_pool(name="ps_pj", bufs=1, space="PSUM") as ps_pj,
              ):
                  XSL = NP // XS
                  ones8 = wp.tile([128, 2, 1], FP8, name="ones8")
                  nc.vector.tensor_copy(ones8[:], ones_r[:, 0:2].rearrange(
                      "p (s o) -> p s o", o=1))
                  NH = NT8 // 2

                  def x8_dma(b, lo, hi):
                      return nc.sync.dma_start(
                          x8t[b][:, lo:hi, :, :],
                          x8d[b].ap()[:, lo:hi, :].rearrange(
                              "p t (s f) -> p t s f", s=2))

                  x8t = [w1p.tile([128, NT8, 2, 128], FP8, name=f"x8_{b}")
                         for b in range(2)]

                  def xt_dma(b2, s):
                      return nc.sync.dma_start(
                          xT[:, b2, s * XSL:(s + 1) * XSL],
                          xTd[:, b2, s * XSL:(s + 1) * XSL])

                  x8_dma(0, 0, NH)
                  x8_dma(0, NH, NT8)
                  for s in range(XS):
                      xt_dma(0, s)
                  x8_dma(1, 0, NH)
                  x8_dma(1, NH, NT8)
                  for s in range(XS):
                      xt_dma(1, s)
                  for b in range(2):
                      # Gram X^T X and colsum X^T 1, both via fp8 DoubleRow
                      g_ps = ps_g.tile([128, 128], F32, tag="G", name=f"G_{b}")
                      cs_ps = ps_g.tile([128, 1], F32, tag="CS", name=f"CS_{b}")
                      for t in range(NT8):
                          nc.tensor.matmul(g_ps[:], x8t[b][:, t, :, :],
                                           x8t[b][:, t, :, :],
                                           start=(t == 0), stop=(t == NT8 - 1),
                                           perf_mode=DR)
                          nc.tensor.matmul(cs_ps[:], x8t[b][:, t, :, :],
                                           ones8[:],
                                           start=(t == 0), stop=(t == NT8 - 1),
                                           perf_mode=DR)
                      # ---- project Gram -> (sum, sumsq) of pre1 ----
                      g_sbuf = w1p.tile([128, 128], F32R, tag="gsb", name=f"gsb_{b}")
                      nc.vector.tensor_copy(g_sbuf[:], g_ps[:])
                      mm1 = ps_pj.tile([128, 512], F32, tag="pj", name=f"mm1_{b}")
                      nc.tensor.matmul(mm1[:], g_sbuf[:], w1_r[b][:], start=True,
                                       stop=True)
                      prod = w1p.tile([128, 512], F32R, tag="prod", name=f"prod_{b}")
                      nc.vector.tensor_mul(prod[:], w1_r[b][:], mm1[:])
                      xsum_r = w1p.tile([128, 4], F32R, tag="xsumr",
                                        name=f"xsumr_{b}")
                      for q in range(4):
                          nc.vector.tensor_copy(xsum_r[:, q:q + 1], cs_ps[:])
                      sq = ps_pj.tile([128, 4, 4], F32, tag="sq", name=f"sq_{b}")
                      sm = ps_pj.tile([128, 4, 4], F32, tag="sm", name=f"sm_{b}")
                      for m in range(4):
                          nc.tensor.matmul(sq[:, m, :], prod[:, ts(m, 128)],
                                           ones_r[:], start=(m == 0),
                                           stop=(m == 3), skip_group_check=True)
                      for m in range(4):
                          nc.tensor.matmul(sm[:, m, :], w1_r[b][:, ts(m, 128)],
                                           xsum_r[:], start=(m == 0),
                                           stop=(m == 3), skip_group_check=True)
                      nc.vector.tensor_copy(pay[0][b][:, :, 1:2], sq[:, :, 0:1])
                      nc.vector.tensor_copy(pay[0][b][:, :, 0:1], sm[:, :, 0:1])
                      issue_allreduce(0, b)
                      load_pass2_weights(b)

              # ================= pass 2: L1 -> BN1 -> L2 -> stats/spill ====
              with (
                  tc.tile_pool(name="w2p", bufs=4) as w2p,
                  tc.tile_pool(name="ps_p1", bufs=4, space="PSUM") as ps_p1,
                  tc.tile_pool(name="ps_p2", bufs=3, space="PSUM") as ps_p2,
              ):
                  h1_tiles = {}

                  def l1_chunk(b, c):
                      """L1 matmuls + BN1+relu for chunk c (issued one chunk
                      ahead of the L2 consumer so the h1 ACTs hide under the
                      previous chunk's L2 matmuls)."""
                      c0, tc_sz = CHUNKS[c]
                      h1 = w2p.tile([128, 4, tc_sz], BF16, tag="h1",
                                    name=f"h1_{c}_{b}")
                      for m in range(4):
                          pp = ps_p1.tile([128, tc_sz], F32, tag="p1",
                                          name=f"p1_{c}_{b}_{m}")
                          nc.tensor.matmul(pp[:], w1_bf[b][:, ts(m, 128)],
                                           xT[:, b, c0:c0 + tc_sz],
                                           start=True, stop=True)
                          nc.scalar.activation(
                              h1[:, m, :], pp[:], AF.Relu,
                              bias=shift_t[0][:, b, m:m + 1],
                              scale=scale_t[0][:, b, m:m + 1])
                      if c == C - 1:
                          # padded nodes: relu(shift) != 0 would pollute BN2 stats
                          nc.scalar.mul(h1[:, :, PAD0:], h1[:, :, PAD0:], 0.0)
                      h1_tiles[c] = h1

                  for b in range(2):
                      finish_stats(0, b)
                      l1_chunk(b, 0)
                      for c, (c0, tc_sz) in enumerate(CHUNKS):
                          if b == 1 and c == 6:
                              # AR(1,0) is long done by now; computing its
                              # scale/shift here keeps pass-3 startup off the
                              # critical path
                              finish_stats(1, 0)
                          if c + 1 < C:
                              l1_chunk(b, c + 1)
                          h1 = h1_tiles.pop(c)
                          spl = w2p.tile([128, 4, tc_sz], BF16, tag="spl",
                                         name=f"spl_{c}_{b}")
                          for m in range(4):
                              pq = ps_p2.tile([128, tc_sz], F32, tag="p2",
                                              name=f"p2_{c}_{b}_{m}")
                              for k in range(4):
                                  nc.tensor.matmul(pq[:], w2_t[b][:, k, ts(m, 128)],
                                                   h1[:, k, :],
                                                   start=(k == 0), stop=(k == 3))
                              # copy frees the PSUM bank; stats read the SBUF
                              # copy and can lag without stalling the PE.
                              # Last chunks go all-DVE so ACT is free to start
                              # pass-3's h2 immediately.
                              if m % 2 == 0 and not (b == 1 and c >= C - 2):
                                  nc.scalar.copy(spl[:, m, :], pq[:])
                              else:
                                  nc.vector.tensor_copy(spl[:, m, :], pq[:])
                          for m in range(4):
                              nc.vector.bn_stats(st2[:, b, m, c, :], spl[:, m, :])
                          nc.sync.dma_start(spill[:, b, :, c0:c0 + tc_sz], spl[:])
                      # ---- aggregate local BN2 stats, launch AllReduce ----
                      agg = stat.tile([128, 4, 2], F32, tag="agg", name=f"agg_{b}")
                      for m in range(4):
                          nc.vector.bn_aggr(agg[:, m, :], st2[:, b, m, :, :])
                      tmp2 = stat.tile([128, 4], F32, tag="tmp2", name=f"tmp2_{b}")
                      nc.vector.tensor_scalar_mul(pay[1][b][:, :, 0], agg[:, :, 0],
                                                  float(NP))
                      nc.vector.tensor_mul(tmp2[:], agg[:, :, 0], agg[:, :, 0])
                      nc.vector.tensor_add(tmp2[:], tmp2[:], agg[:, :, 1])
                      nc.vector.tensor_scalar_mul(pay[1][b][:, :, 1], tmp2[:],
                                                  float(NP))
                      issue_allreduce(1, b)
                      if b == 0:
                          load_pass3_weights()
                          # prefetch first b0 spill chunks on the gpsimd
                          # SWDGE queue (drains right after AR(1,0)), so
                          # pass-3 b0 sections can run during AR(1,1)
                          for cq in range(PF3):
                              cq0, cqs = CHUNKS[cq]
                              pftile = pf3.tile([128, 4, cqs], BF16, tag="pf",
                                                name=f"pf3_{cq}")
                              nc.gpsimd.dma_start(
                                  pftile[:], spill[:, 0, :, cq0:cq0 + cqs])
                              pf3_tiles[cq] = pftile
                          load_stats(1, 0, dma_engine=nc.gpsimd)

            # ================= pass 3: BN2 -> Lf -> fp8 head =============
            with (
                tc.tile_pool(name="w3p", bufs=2) as w3p,
                tc.tile_pool(name="ps_f", bufs=4, space="PSUM") as ps_f,
                tc.tile_pool(name="ps_t", bufs=4, space="PSUM") as ps_t,
            ):
                ps_o = ps_t  # share the t/o banks (tag-separated slots share pool)

                sec_h2 = {}
                f_tiles = {}
                t_tiles = {}

                def prep(c, b):
                    """Spill reload + BN2+relu, issued one section ahead so the
                    h2 ACTs hide under the previous section's Lf matmuls."""
                    c0, tc_sz = CHUNKS[c]
                    if b == 0 and c in pf3_tiles:
                        pre2 = pf3_tiles.pop(c)
                    else:
                        pre2 = w3p.tile([128, 4, tc_sz], BF16, tag="pre2ld",
                                        bufs=4, name=f"pre2_{c}_{b}")
                        nc.sync.dma_start(pre2[:], spill[:, b, :, c0:c0 + tc_sz])
                    h2 = w3p.tile([128, 4, tc_sz], BF16, tag="h2", bufs=3,
                                  name=f"h2_{c}_{b}")
                    for k in range(4):
                        nc.scalar.activation(
                            h2[:, k, :], pre2[:, k, :], AF.Relu,
                            bias=shift_t[1][:, b, k:k + 1],
                            scale=scale_t[1][:, b, k:k + 1])
                    sec_h2[(c, b)] = h2

                def comp(c, b):
                    """Lf -> fp8 hi/lo split of f for branch b."""
                    c0, tc_sz = CHUNKS[c]
                    h2 = sec_h2.pop((c, b))
                    fhi = w3p.tile([128, 4, tc_sz], FP8, tag=f"fhi{b}",
                                   bufs=(PF3 + 2 if b == 0 else 2),
                                   name=f"fhi{b}_{c}")
                    flo = w3p.tile([128, 4, tc_sz], FP8, tag=f"flo{b}",
                                   bufs=(PF3 + 2 if b == 0 else 2),
                                   name=f"flo{b}_{c}")
                    f_tiles[(c, b)] = (fhi, flo)
                    for m in range(4):
                        pf = ps_f.tile([128, tc_sz], F32, tag="f",
                                       name=f"pf_{c}_{b}_{m}")
                        for k in range(4):
                            nc.tensor.matmul(pf[:], wf_t[b][:, k, ts(m, 128)],
                                             h2[:, k, :],
                                             start=(k == 0), stop=(k == 3))
                        # split hi-copies between ACT and DVE so neither
                        # engine paces the (DVE-heavy) f-split
                        if b == 0 and m % 2 == 0:
                            nc.scalar.copy(fhi[:, m, :], pf[:])
                        else:
                            nc.vector.tensor_copy(fhi[:, m, :], pf[:])
                        nc.vector.scalar_tensor_tensor(
                            flo[:, m, :], pf[:], 1.0, fhi[:, m, :],
                            op0=ALU.mult, op1=ALU.subtract)

                def head(c):
                    """Wh1 fp8 DoubleRow 3-term + t ACTs (Wh2 deferred)."""
                    c0, tc_sz = CHUNKS[c]
                    f0hi, f0lo = f_tiles.pop((c, 0))
                    f1hi, f1lo = f_tiles.pop((c, 1))
                    t_sb = w3p.tile([128, 4, tc_sz], BF16, tag="t_sb", bufs=3,
                                    name=f"t_sb_{c}")
                    t_tiles[c] = t_sb
                    for m in range(4):
                        ptl = ps_t.tile([128, tc_sz], F32, tag="t",
                                        name=f"ptl_{c}_{m}")
                        n_mm = 0
                        for fh, fl, kb in ((f0hi, f0lo, 0), (f1hi, f1lo, 4)):
                            for j in range(2):
                                wsl_h = wh1h[:, kb + 2 * j:kb + 2 * j + 2, ts(m, 128)]
                                wsl_l = wh1l[:, kb + 2 * j:kb + 2 * j + 2, ts(m, 128)]
                                fsl_h = fh[:, 2 * j:2 * j + 2, :]
                                fsl_l = fl[:, 2 * j:2 * j + 2, :]
                                for wsl, fsl in ((wsl_h, fsl_h), (wsl_h, fsl_l),
                                                 (wsl_l, fsl_h)):
                                    nc.tensor.matmul(ptl[:], wsl, fsl,
                                                     start=(n_mm == 0),
                                                     stop=(n_mm == 11),
                                                     perf_mode=DR)
                                    n_mm += 1
                        nc.scalar.activation(t_sb[:, m, :], ptl[:], AF.Relu,
                                             bias=bh1_sb[:, m:m + 1],
                                             scale=1.0 / 64.0)

                def wh2_out(c):
                    c0, tc_sz = CHUNKS[c]
                    t_sb = t_tiles.pop(c)
                    po = ps_o.tile([10, tc_sz], F32, tag="t", name=f"po_{c}")
                    for k in range(4):
                        nc.tensor.matmul(po[:], wh2_t[:, k, :], t_sb[:, k, :],
                                         start=(k == 0), stop=(k == 3))
                    o_sb = w3p.tile([10, tc_sz], F32, tag="o_sb", name=f"o_sb_{c}")
                    nc.scalar.activation(o_sb[:], po[:], AF.Identity,
                                         bias=bh2_sb[:, 0:1])
                    nc.sync.dma_start(outd[:, c0:c0 + tc_sz], o_sb[:])

                # section order: b0 run-ahead (hides AllReduce(1,1)), then
                # interleaved (c,0)/(c,1) with head after each (c,1)
                SECS = [(c, 0) for c in range(PF3)]
                for c in range(C):
                    if c >= PF3:
                        SECS.append((c, 0))
                    SECS.append((c, 1))

                prep(*SECS[0])
                pend = None
                for i, s in enumerate(SECS):
                    if i + 1 < len(SECS):
                        prep(*SECS[i + 1])
                    comp(*s)
                    if s == (5, 0):
                        finish_stats(1, 1)
                    if s[1] == 1:
                        if pend is not None:
                            wh2_out(pend)
                        head(s[0])
                        pend = s[0]
                wh2_out(pend)

    nc.compile()
    return nc


def _get_program():
    if "nc" not in _CACHE:
        _CACHE["nc"] = _build_program()
    return _CACHE["nc"]


def kernel(**inputs):
    nc = _get_program()
    F8 = ml_dtypes.float8_e4m3
    B16 = ml_dtypes.bfloat16

    def shard_pad(x):
        x = np.ascontiguousarray(x, dtype=np.float32).reshape(NCORES, NSH, 128)
        pad = np.zeros((NCORES, NP - NSH, 128), dtype=np.float32)
        return np.concatenate([x, pad], axis=1)  # [NCORES, NP, 128]

    xp = [shard_pad(inputs["x_1"]), shard_pad(inputs["x_2"])]
    # xT: [NCORES, 128, 2, NP] bf16
    xT = np.stack([np.swapaxes(xp[0], 1, 2), np.swapaxes(xp[1], 1, 2)],
                  axis=1).astype(B16)  # [NCORES, 2, 128, NP]
    xT = np.ascontiguousarray(np.swapaxes(xT, 1, 2))  # [NCORES, 128, 2, NP]
    # xin8: [NCORES, 128, NT8, 256] fp8 per branch (node-within-tile on the
    # partition axis, contiguous per partition row for fast DMA)
    x8 = [np.ascontiguousarray(
              xp[b].reshape(NCORES, NT8, 2, 128, 128).transpose(0, 3, 1, 2, 4)
              .reshape(NCORES, 128, NT8, 256)).astype(F8)
          for b in range(2)]

    rep = {}
    # W1 stacked [128, 2, 512]
    rep["W1S"] = np.ascontiguousarray(
        np.stack([inputs["W1_1"], inputs["W1_2"]], axis=1), dtype=np.float32)
    for nm in ("W2_1", "W2_2", "Wf_1", "Wf_2"):
        rep[nm] = np.ascontiguousarray(inputs[nm]).astype(B16)
    rep["Wh2"] = np.ascontiguousarray(inputs["Wh2"]).astype(B16)
    for nm in ("g1_1", "be1_1", "g2_1", "be2_1",
               "g1_2", "be1_2", "g2_2", "be2_2", "bh2"):
        rep[nm] = np.ascontiguousarray(inputs[nm], dtype=np.float32)

    # Wh1 -> x64 fp8 hi/lo pair in [128, 8, 512] (p k m) layout
    wh1 = np.ascontiguousarray(inputs["Wh1"], dtype=np.float32) * 64.0
    wh1_pkm = wh1.reshape(8, 128, 512).swapaxes(0, 1)  # [128, 8, 512]
    whi = wh1_pkm.astype(F8)
    wlo = (wh1_pkm - whi.astype(np.float32)).astype(F8)
    rep["WH1HI"] = np.ascontiguousarray(whi)
    rep["WH1LO"] = np.ascontiguousarray(wlo)
    # bh1' = bh1 + Wh1^T [bf_1; bf_2]  (absorbs the Lf biases)
    bfcat = np.concatenate([np.asarray(inputs["bf_1"], np.float64),
                            np.asarray(inputs["bf_2"], np.float64)])
    rep["BH1P"] = (np.asarray(inputs["bh1"], np.float64)
                   + bfcat @ np.asarray(inputs["Wh1"], np.float64)).astype(np.float32)

    rep["AUX"] = np.ones((128, 4), dtype=np.float32)
    rep["EPSA"] = np.full((128, 1), EPS, dtype=np.float32)

    in_maps = []
    for c in range(NCORES):
        m = {"XT": xT[c], "XIN8_1": x8[0][c], "XIN8_2": x8[1][c]}
        m.update(rep)
        in_maps.append(m)

    res = bass_utils.run_bass_kernel_spmd(nc, in_maps, core_ids=list(range(NCORES)))
    parts = [res.results[c]["OUT"][:, :NSH] for c in range(NCORES)]
    out = np.concatenate(parts, axis=1).T
    return np.ascontiguousarray(out, dtype=np.float32)
